# revision 15
# baseline (speedup 1.0000x reference)
"""BUIR (LightGCN x2 + predictor) Trainium2 kernel — 8 NeuronCores, SPMD.

v5 = v3 two-phase SpMM + batch-driven layer pruning:
  - Outputs only need acc = ego+y1+y2+y3 at ~8k batch nodes. Backward
    reachability: layer-3 rows = batch nodes (172k edges, 2.7%), layer-2
    rows = cols(E3)+batch (2.9M edges, 45%), layer-1 full. Each layer
    gets its own phase-1 plan (cells/fills/quads) over its edge subset.
  - Phase-2 for layers 2/3 runs on packed 128-row fills of ACTIVE rows
    only; y rows land via indirect scatter-DMA (positions per fill).
    Layer-1 phase-2 unchanged (dense affine writes).
  - acc tensor + per-fill accumulate-DMAs are gone: the epilogue gathers
    ego/y1/y2/y3 at batch positions, folds them on DVE, then applies the
    predictor. All tables stay [S,128] per core (no packing), so col
    indexing and the AllGathers are identical to v3.
"""

import os
import sys

sys.path.insert(0, "/opt/trn_rl_repo")

import numpy as np
import ml_dtypes

U, NI, D = 100000, 200000, 64
N = U + NI
NNZ = 6400000
B = 4096
L = 3
NCORES = 8
RPC = N // NCORES  # 37500
WIN = 32768
B_ROWS = 2560
NBLK = -(-RPC // B_ROWS)  # 15
W1 = (1, 2, 3, 4, 6, 8, 12, 16)
NW1 = len(W1)
R1 = tuple(128 // w for w in W1)
RSLOT = (128, 64, 64, 32, 32, 32, 32, 32)
PERFILL = (1, 2, 2, 4, 4, 4, 4, 4)
W2 = 10  # phase-2 lanes per row (one per chunk)
GT1 = 48
GT2 = 64

_W1LUT = np.zeros(200, np.int64)
for _d in range(1, 200):
    _W1LUT[_d] = next((i for i, w in enumerate(W1) if _d <= w), NW1 - 1)

# static full phase-2 layout (layer 1 only)
_P2_BLOCKS = []
_S = 0
for _b in range(NBLK):
    _rows = min(B_ROWS, RPC - _b * B_ROWS)
    _fills = -(-_rows // 128)
    _P2_BLOCKS.append(
        dict(rows=_rows, fills=_fills, tiles=10 * _fills, ybase=_S, tbase=0)
    )
    _S += 128 * _fills
_tb = 0
for _bl in _P2_BLOCKS:
    _bl["tbase"] = _tb
    _tb += _bl["tiles"]
S = _S  # 37504
NP = NCORES * S
NCH = -(-NP // WIN)
assert NCH <= W2, (NCH, W2)
NT2 = _tb


# ---------------- planner ----------------


def _plan_phase1(adj_row, adj_col, adj_val, mask):
    """Phase-1 plan for one edge subset. Returns dict with the program
    structure (max envelope across cores) + per-core gidx/val/wpos."""
    adj_row = adj_row[mask]
    adj_col = adj_col[mask]
    adj_val = adj_val[mask]
    core_n = adj_col // RPC
    sig_col = core_n * S + (adj_col - core_n * RPC)
    ch_all = sig_col // WIN
    widx_all = sig_col - ch_all * WIN
    core_all = adj_row // RPC

    percore = []
    for k in range(NCORES):
        m = core_all == k
        lr = adj_row[m] - k * RPC
        ch = ch_all[m]
        wx = widx_all[m]
        v = adj_val[m]
        key = lr * NCH + ch
        srt = np.argsort(key, kind="stable")
        lr, ch, wx, v, key = lr[srt], ch[srt], wx[srt], v[srt], key[srt]
        cnt = np.bincount(key, minlength=RPC * NCH)
        gkey = np.nonzero(cnt)[0]
        gdeg = cnt[gkey]
        if len(gdeg):
            assert gdeg.max() <= W1[-1], gdeg.max()
        g_wi = _W1LUT[gdeg] if len(gdeg) else gdeg
        g_b = (gkey // NCH) // B_ROWS
        cell = (g_b * NW1 + g_wi) * NCH + gkey % NCH
        gsrt = np.argsort(cell, kind="stable")
        percore.append(
            dict(wx=wx, v=v, key=key, cnt=cnt, gkey=gkey, cell=cell, gsrt=gsrt)
        )

    NCELL = NBLK * NW1 * NCH
    cell_tiles = np.zeros(NCELL, np.int64)
    wi_of_cell = (np.arange(NCELL) // NCH) % NW1
    r1_of_cell = np.array(R1)[wi_of_cell]
    for pc in percore:
        c_cnt = np.bincount(pc["cell"], minlength=NCELL)
        cell_tiles = np.maximum(cell_tiles, -(-c_cnt // r1_of_cell))
    cell_tiles = cell_tiles.reshape(NBLK, NW1, NCH)
    for b in range(NBLK):
        for wi in range(NW1):
            pad = (-cell_tiles[b, wi].sum()) % 4
            if pad:
                cell_tiles[b, wi, NCH - 1] += pad
    cell_tiles = cell_tiles.reshape(NCELL)
    cell_t0 = np.concatenate([[0], np.cumsum(cell_tiles)])[:-1]
    NT1 = int(cell_tiles.sum())
    if NT1 == 0:
        NT1 = 4
        cell_tiles[NCH - 1] = 4
        cell_t0 = np.concatenate([[0], np.cumsum(cell_tiles)])[:-1]

    tile_b = np.zeros(NT1, np.int64)
    tile_wi = np.zeros(NT1, np.int64)
    tile_ch = np.zeros(NT1, np.int64)
    for c in range(NCELL):
        t0, nt = cell_t0[c], cell_tiles[c]
        if nt:
            tile_b[t0 : t0 + nt] = c // (NW1 * NCH)
            tile_wi[t0 : t0 + nt] = (c // NCH) % NW1
            tile_ch[t0 : t0 + nt] = c % NCH
    NQ1 = NT1 // 4
    quad_b = tile_b[::4]
    quad_wi = tile_wi[::4]
    quad_R = np.array(R1)[quad_wi]
    assert (tile_wi[::4] == tile_wi[3::4]).all()
    assert (tile_b[::4] == tile_b[3::4]).all()

    fills1 = []  # (q0, nq, b, pbase)
    quad_p0 = np.zeros(NQ1, np.int64)
    quad_fill = np.zeros(NQ1, np.int64)
    win_used = np.ones(NBLK, np.int64)  # slot 0 = zeros
    q = 0
    while q < NQ1:
        b, wi = int(quad_b[q]), int(quad_wi[q])
        nq = 1
        while (
            nq < PERFILL[wi]
            and q + nq < NQ1
            and quad_b[q + nq] == b
            and quad_wi[q + nq] == wi
        ):
            nq += 1
        for i in range(nq):
            quad_p0[q + i] = i * RSLOT[wi]
            quad_fill[q + i] = len(fills1)
        fills1.append((q, nq, b, int(win_used[b])))
        win_used[b] += 512
        q += nq
    assert win_used.max() <= WIN, win_used.max()
    fill_pbase = np.array([f[3] for f in fills1], np.int64)

    quad_of_tile = np.arange(NT1) // 4
    cblk_of_tile = np.arange(NT1) % 4

    gidx1, val1, wpos_l = [], [], []
    for k, pc in enumerate(percore):
        gsrt = pc["gsrt"]
        csort = pc["cell"][gsrt]
        cstart = np.concatenate(
            [[0], np.cumsum(np.bincount(csort, minlength=NCELL))]
        )
        seq = np.arange(len(csort)) - cstart[csort]
        r1c = r1_of_cell[csort]
        g_tile = cell_t0[csort] + seq // r1c
        g_slot = seq % r1c
        assert (seq // r1c < cell_tiles[csort]).all()
        g_w = np.array(W1)[(csort // NCH) % NW1]
        g_lane0 = g_slot * g_w
        gq = quad_of_tile[g_tile]
        gpos = (
            fill_pbase[quad_fill[gq]]
            + cblk_of_tile[g_tile] * 128
            + quad_p0[gq]
            + g_slot
        )
        if len(gpos):
            assert gpos.max() < WIN
        ppos = np.zeros(RPC * NCH, np.int64)
        ppos[pc["gkey"][gsrt]] = gpos
        tile_of_g = np.zeros(RPC * NCH, np.int64)
        lane0_of_g = np.zeros(RPC * NCH, np.int64)
        tile_of_g[pc["gkey"][gsrt]] = g_tile
        lane0_of_g[pc["gkey"][gsrt]] = g_lane0
        key = pc["key"]
        kstart = np.concatenate([[0], np.cumsum(pc["cnt"])])
        rank = np.arange(len(key)) - kstart[key]
        e_tile = tile_of_g[key]
        e_lane = lane0_of_g[key] + rank
        gi = np.zeros((128, NT1), np.int16)
        va = np.zeros((128, NT1), np.float32)
        gi[e_lane, e_tile] = pc["wx"].astype(np.int16)
        va[e_lane, e_tile] = pc["v"]
        gidx1.append(gi)
        val1.append(va)
        wpos_l.append(ppos.reshape(RPC, NCH))

    calls1 = []
    for c in range(NCELL):
        if cell_tiles[c]:
            calls1.append((int(cell_t0[c]), int(cell_tiles[c]), int(c % NCH)))

    return dict(
        NT1=NT1,
        NQ1=NQ1,
        fills1=fills1,
        quad_p0=quad_p0,
        quad_R=quad_R,
        quad_wi=quad_wi,
        calls1=calls1,
        tile_ch=tile_ch,
        gidx1=gidx1,
        val1=val1,
        wpos=wpos_l,
    )


def _plan_phase2_full(pl):
    """Full-coverage phase-2 idx (layer 1): g2idx per core [128, NT2]."""
    gidx2 = []
    t_all = np.arange(NT2)
    blk_of_t = np.zeros(NT2, np.int64)
    for bi, bl in enumerate(_P2_BLOCKS):
        blk_of_t[bl["tbase"] : bl["tbase"] + bl["tiles"]] = bi
    tb_arr = np.array([bl["tbase"] for bl in _P2_BLOCKS])
    rows_b = np.array([bl["rows"] for bl in _P2_BLOCKS])
    for k in range(NCORES):
        wpos = pl["wpos"][k]
        lanes = (
            (t_all - tb_arr[blk_of_t])[None, :] * 128
            + np.arange(128)[:, None]
        )
        row_in_b = lanes // W2
        s_of = lanes % W2
        r_glob = blk_of_t[None, :] * B_ROWS + row_in_b
        valid = (row_in_b < rows_b[blk_of_t][None, :]) & (r_glob < RPC)
        rv = np.where(valid, r_glob, 0)
        g2v = wpos[rv, s_of]
        gidx2.append(np.where(valid, g2v, 0).astype(np.int16))
    return gidx2


def _plan_phase2_packed(pl, act_percore):
    """Packed phase-2 for a pruned layer: fills of 128 ACTIVE rows per
    B_ROWS block; returns program structure + per-core g2 idx and scatter
    positions. Structure (fills per block) is the max envelope over cores."""
    # rows per (core, block)
    rows_cb = [
        [a[(a >= b * B_ROWS) & (a < (b + 1) * B_ROWS)] for b in range(NBLK)]
        for a in act_percore
    ]
    blocks = []  # (b2, fills, tbase, fbase)
    tb = 0
    fb = 0
    for b in range(NBLK):
        mx = max(len(rows_cb[k][b]) for k in range(NCORES))
        fills = -(-mx // 128) if mx else 0
        if fills:
            blocks.append(dict(b2=b, fills=fills, tbase=tb, fbase=fb))
            tb += 10 * fills
            fb += fills
    NT2p = tb
    NF = fb
    g2 = [np.zeros((128, max(NT2p, 1)), np.int16) for _ in range(NCORES)]
    ypos = [
        np.full((128, max(NF, 1)), RPC, np.int32) for _ in range(NCORES)
    ]  # default: dump row
    for k in range(NCORES):
        wpos = pl["wpos"][k]
        for bl in blocks:
            rows = rows_cb[k][bl["b2"]]
            for f in range(bl["fills"]):
                r128 = rows[f * 128 : (f + 1) * 128]
                nr = len(r128)
                if nr == 0:
                    continue
                ypos[k][:nr, bl["fbase"] + f] = r128.astype(np.int32)
                for t in range(10):
                    fl = t * 128 + np.arange(128)
                    ri = (f * 1280 + fl) // 10 - f * 128
                    s = (f * 1280 + fl) % 10
                    ok = ri < nr
                    rv = np.where(ok, r128[np.minimum(ri, max(nr - 1, 0))], 0)
                    gv = wpos[rv, s]
                    g2[k][:, bl["tbase"] + f * 10 + t] = np.where(
                        ok, gv, 0
                    ).astype(np.int16)
    return dict(blocks=blocks, NT2=NT2p, NF=NF, g2=g2, ypos=ypos)


def _plan(adj_row, adj_col, adj_val, user_idx, item_idx):
    adj_row = np.asarray(adj_row, np.int64)
    adj_col = np.asarray(adj_col, np.int64)
    adj_val = np.asarray(adj_val, np.float32)
    batch = np.unique(
        np.concatenate(
            [np.asarray(user_idx, np.int64), U + np.asarray(item_idx, np.int64)]
        )
    )
    m3 = np.zeros(N, bool)
    m3[batch] = True
    e3 = m3[adj_row]
    a2 = m3.copy()
    a2[adj_col[e3]] = True
    e2 = a2[adj_row]
    full = np.ones(NNZ, bool)

    pl0 = _plan_phase1(adj_row, adj_col, adj_val, full)
    pl1 = _plan_phase1(adj_row, adj_col, adj_val, e2)
    pl2 = _plan_phase1(adj_row, adj_col, adj_val, e3)

    gidx2_full = _plan_phase2_full(pl0)

    def act_percore(mask):
        out = []
        nodes = np.nonzero(mask)[0]
        for k in range(NCORES):
            mm = (nodes >= k * RPC) & (nodes < (k + 1) * RPC)
            out.append(np.sort(nodes[mm] - k * RPC))
        return out

    p2_1 = _plan_phase2_packed(pl1, act_percore(a2))
    p2_2 = _plan_phase2_packed(pl2, act_percore(m3))

    return dict(
        layers=[pl0, pl1, pl2],
        gidx2_full=gidx2_full,
        p2p=[None, p2_1, p2_2],
    )


def _plan_batch(plan, user_idx, item_idx):
    out = {}
    counts = []
    for name, nodes in (
        ("u", np.asarray(user_idx, np.int64)),
        ("i", U + np.asarray(item_idx, np.int64)),
    ):
        owner = nodes // RPC
        loc = nodes - owner * RPC
        out[name] = (owner, loc)
        counts.append(np.bincount(owner, minlength=NCORES).max())
    BMAX = int(-(-max(counts) // 128) * 128)
    plan["BMAX"], plan["NBB"] = BMAX, BMAX // 128
    for name in ("u", "i"):
        owner, loc = out[name]
        pos_arr = np.zeros((NCORES, BMAX), np.int64)
        slot = np.zeros(B, np.int64)
        for k in range(NCORES):
            m = np.nonzero(owner == k)[0]
            pos_arr[k, : len(m)] = loc[m]
            slot[m] = np.arange(len(m))
        plan[f"b{name}_pos"] = [
            pos_arr[k].reshape(BMAX // 128, 128).T.astype(np.int32).copy()
            for k in range(NCORES)
        ]
        plan[f"b{name}_owner"] = owner
        plan[f"b{name}_slot"] = slot
    return plan


# ---------------- bass program ----------------


def _lhs_layout():
    base1 = {}
    off = 0
    for wi in range(NW1):
        base1[wi] = off
        off += R1[wi]
    base2 = off  # 5 patterns of 64 cols each
    return base1, base2, off + 5 * 64


def _phase1_segs(pl):
    segs = []
    for t0, nt, ch in pl["calls1"]:
        t = t0
        while t < t0 + nt:
            g = t // GT1
            n = min(t0 + nt, (g + 1) * GT1) - t
            segs.append((g, t - g * GT1, n, ch))
            t += n
    segs.sort(key=lambda s: (s[0], s[1]))
    by_g = {}
    for s in segs:
        by_g.setdefault(s[0], []).append(s)
    return by_g


def _build_program(plan):
    ablate = set(os.environ.get("BUIR_ABLATE", "").split(","))
    import concourse.bass as bass
    import concourse.bacc as bacc
    import concourse.mybir as mybir
    import concourse.tile as tile

    f32, bf16, i16, i32 = (
        mybir.dt.float32,
        mybir.dt.bfloat16,
        mybir.dt.int16,
        mybir.dt.int32,
    )
    NBB = plan["NBB"]
    base1, base2, RSUM = _lhs_layout()
    layers = plan["layers"]
    segs_by_layer = [_phase1_segs(pl) for pl in layers]

    nc = bacc.Bacc(
        "TRN2",
        target_bir_lowering=False,
        debug=False,
        num_devices=NCORES,
        num_swdge_queues=4,
    )
    qctr = [0]

    def next_q():
        qctr[0] = (qctr[0] + 1) % 4
        return qctr[0]
    x2_0 = nc.declare_dram_parameter("x2_0", [NP, 128], bf16, isOutput=False)
    ego = nc.declare_dram_parameter("ego", [S, 128], f32, isOutput=False)
    g1p, v1p = [], []
    for l in range(L):
        nt = layers[l]["NT1"]
        g1p.append(
            nc.declare_dram_parameter(f"g1idx_{l}", [128, nt * 8], i16, isOutput=False)
        )
        v1p.append(
            nc.declare_dram_parameter(f"val1_{l}", [128, nt], bf16, isOutput=False)
        )
    g2idx0 = nc.declare_dram_parameter("g2idx0", [128, NT2 * 8], i16, isOutput=False)
    g2p, ypp = [None], [None]
    for l in (1, 2):
        p2 = plan["p2p"][l]
        g2p.append(
            nc.declare_dram_parameter(
                f"g2idx_{l}", [128, max(p2["NT2"], 1) * 8], i16, isOutput=False
            )
        )
        ypp.append(
            nc.declare_dram_parameter(
                f"ypos_{l}", [128, max(p2["NF"], 1)], i32, isOutput=False
            )
        )
    lhs = nc.declare_dram_parameter("lhs", [128, RSUM], bf16, isOutput=False)
    w4t = nc.declare_dram_parameter("w4t", [64, 64], f32, isOutput=False)
    brep = nc.declare_dram_parameter("brep", [128, 64], f32, isOutput=False)
    ident = nc.declare_dram_parameter("ident", [128, 128], f32, isOutput=False)
    bupos = nc.declare_dram_parameter("bupos", [128, NBB], i32, isOutput=False)
    bipos = nc.declare_dram_parameter("bipos", [128, NBB], i32, isOutput=False)
    outs = {}
    for nm in ("u_pred", "u_target", "i_pred", "i_target"):
        outs[nm] = nc.declare_dram_parameter(nm, [plan["BMAX"], 64], f32, isOutput=True)

    with tile.TileContext(nc) as tc:
        with (
            tc.tile_pool(name="const", bufs=1) as cp,
            tc.tile_pool(name="g1", bufs=5) as pg1,
            tc.tile_pool(name="pr", bufs=3) as ppr,
            tc.tile_pool(name="g2", bufs=3) as pg2,
            tc.tile_pool(name="ix", bufs=4) as pix,
            tc.tile_pool(name="yo", bufs=4) as pyo,
            tc.tile_pool(name="ps", bufs=4, space="PSUM") as pps,
            tc.tile_pool(name="dram", bufs=1, space="DRAM") as dp,
        ):
            val1_t = []
            for l in range(L):
                vt = cp.tile([128, layers[l]["NT1"]], bf16)
                nc.sync.dma_start(out=vt[:], in_=v1p[l][:])
                val1_t.append(vt)
            lhs_t = cp.tile([128, RSUM], bf16)
            w4t_t = cp.tile([64, 64], f32)
            brep_t = cp.tile([128, 64], f32)
            ident_t = cp.tile([128, 128], f32)
            bu_t = cp.tile([128, NBB], i32)
            bi_t = cp.tile([128, NBB], i32)
            yp_t = [None]
            for l in (1, 2):
                t = cp.tile([128, max(plan["p2p"][l]["NF"], 1)], i32)
                nc.sync.dma_start(out=t[:], in_=ypp[l][:])
                yp_t.append(t)
            zrow = cp.tile([1, 128], bf16)
            nc.sync.dma_start(out=lhs_t[:], in_=lhs[:])
            nc.sync.dma_start(out=w4t_t[:], in_=w4t[:])
            nc.sync.dma_start(out=brep_t[:], in_=brep[:])
            nc.sync.dma_start(out=ident_t[:], in_=ident[:])
            nc.sync.dma_start(out=bu_t[:], in_=bupos[:])
            nc.sync.dma_start(out=bi_t[:], in_=bipos[:])
            nc.vector.memzero(zrow[:])

            y1 = dp.tile([S, 128], bf16, name="y1")
            y2 = dp.tile([S, 128], bf16, name="y2")
            y3 = dp.tile([S, 128], bf16, name="y3")
            x2_1 = dp.tile([NP, 128], bf16, addr_space="Shared", name="x2_1")
            x2_2 = dp.tile([NP, 128], bf16, addr_space="Shared", name="x2_2")
            part = dp.tile([NBLK * WIN, 128], bf16, name="part")

            for b in range(NBLK):
                nc.sync.dma_start(out=part[b * WIN : b * WIN + 1, :], in_=zrow[:])

            srcs = [x2_0, x2_1, x2_2]
            ydst = [y1, y2, y3]
            nlayers = 1 if "l1" in ablate else L
            for layer in range(nlayers):
                pl = layers[layer]
                NT1 = pl["NT1"]
                lsrc = srcs[layer]
                segs1_by_group = segs_by_layer[layer]
                qp0 = pl["quad_p0"]
                qR = pl["quad_R"]
                qwi = pl["quad_wi"]
                # ---- phase 1 ----
                prod_of = {}
                nextg = [0]

                def issue_group1(
                    g,
                    lname=f"l{layer}",
                    pl=pl,
                    NT1=NT1,
                    lsrc=lsrc,
                    segs1_by_group=segs1_by_group,
                    prod_of=prod_of,
                    layer=layer,
                ):
                    gsz = min(GT1, NT1 - g * GT1)
                    ixt = pix.tile(
                        [128, GT1 * 8], i16, tag="ix1", name=f"ix1_{lname}_{g}"
                    )
                    nc.sync.dma_start(
                        out=ixt[:, : gsz * 8],
                        in_=g1p[layer][:, g * GT1 * 8 : (g * GT1 + gsz) * 8],
                    )
                    gath = pg1.tile(
                        [128, GT1, 128], bf16, tag="g1", name=f"g1_{lname}_{g}"
                    )
                    for _, lo, nt, ch in segs1_by_group.get(g, []):
                        wrows = min(WIN, NP - ch * WIN)
                        nc.gpsimd.dma_gather(
                            out_ap=gath[:, lo : lo + nt, :],
                            in_ap=lsrc[ch * WIN : ch * WIN + wrows, :],
                            idxs_ap=ixt[:, lo * 8 : (lo + nt) * 8],
                            num_idxs=nt * 128,
                            num_idxs_reg=nt * 128,
                            elem_size=128,
                            single_packet=False,
                            queue_num=next_q(),
                        )
                    prod = ppr.tile(
                        [128, GT1 * 128], bf16, tag="pr", name=f"pr_{lname}_{g}"
                    )
                    nc.vector.tensor_tensor(
                        out=prod[:, : gsz * 128].rearrange(
                            "p (g d) -> p g d", g=gsz
                        ),
                        in0=gath[:, :gsz, :],
                        in1=val1_t[layer][:, g * GT1 : g * GT1 + gsz]
                        .unsqueeze(2)
                        .to_broadcast([128, gsz, 128]),
                        op=mybir.AluOpType.mult,
                    )
                    prod_of[g] = prod
                    prod_of.pop(g - 3, None)

                for fi, (q0, nq, fb, pbase) in enumerate(
                    [] if "nop1" in ablate else pl["fills1"]
                ):
                    ps = pps.tile([128, 512], f32, tag="ps", name=f"p1_{layer}_{fi}")
                    for q in range(q0, q0 + nq):
                        g = (4 * q) // GT1
                        while nextg[0] <= g:
                            issue_group1(nextg[0])
                            nextg[0] += 1
                        lo = 4 * q - g * GT1
                        nc.tensor.matmul(
                            out=ps[int(qp0[q]) : int(qp0[q] + qR[q]), :],
                            lhsT=lhs_t[
                                :, base1[int(qwi[q])] : base1[int(qwi[q])] + int(qR[q])
                            ],
                            rhs=prod_of[g][:, lo * 128 : (lo + 4) * 128],
                            start=True,
                            stop=True,
                            tile_position=(0, int(qp0[q])),
                        )
                    pb = pyo.tile([128, 512], bf16, tag="p1o", name=f"po_{layer}_{fi}")
                    nc.vector.tensor_copy(out=pb[:], in_=ps[:])
                    dst = part[
                        fb * WIN + pbase : fb * WIN + pbase + 512, :
                    ].rearrange("(c p) d -> p c d", c=4)
                    nc.sync.dma_start(
                        out=dst, in_=pb[:].rearrange("p (c d) -> p c d", c=4)
                    )

                # ---- phase 2 ----
                ydram = ydst[layer]
                if layer == 0:
                    for b2, bl in enumerate([] if "nop2" in ablate else _P2_BLOCKS):
                        ntil = bl["tiles"]
                        g2_of = {}
                        ng2done = [0]

                        def issue_group2(g, b2=b2, bl=bl, ntil=ntil, g2_of=g2_of):
                            gsz = min(GT2, ntil - g * GT2)
                            ixt = pix.tile(
                                [128, GT2 * 8], i16, tag="ix2",
                                name=f"ix2_{layer}_{b2}_{g}",
                            )
                            nc.sync.dma_start(
                                out=ixt[:, : gsz * 8],
                                in_=g2idx0[
                                    :,
                                    (bl["tbase"] + g * GT2) * 8 : (
                                        bl["tbase"] + g * GT2 + gsz
                                    )
                                    * 8,
                                ],
                            )
                            gt = pg2.tile(
                                [128, GT2, 128], bf16, tag="g2",
                                name=f"g2_{layer}_{b2}_{g}",
                            )
                            nc.gpsimd.dma_gather(
                                out_ap=gt[:, :gsz, :],
                                in_ap=part[b2 * WIN : (b2 + 1) * WIN, :],
                                idxs_ap=ixt[:, : gsz * 8],
                                num_idxs=gsz * 128,
                                num_idxs_reg=gsz * 128,
                                elem_size=128,
                                single_packet=False,
                                queue_num=next_q(),
                            )
                            g2_of[g] = gt
                            g2_of.pop(g - 3, None)

                        for f in range(bl["fills"]):
                            ps = pps.tile(
                                [128, 128], f32, tag="ps2", bufs=2,
                                name=f"p2_{layer}_{b2}_{f}",
                            )
                            for t in range(10):
                                tloc = 10 * f + t
                                g = tloc // GT2
                                while ng2done[0] <= g:
                                    issue_group2(ng2done[0])
                                    ng2done[0] += 1
                                lo = tloc - g * GT2
                                half = t // 5
                                pat = t % 5
                                nc.tensor.matmul(
                                    out=ps[half * 64 : half * 64 + 64, :],
                                    lhsT=lhs_t[
                                        :, base2 + pat * 64 : base2 + pat * 64 + 64
                                    ],
                                    rhs=g2_of[g][:, lo, :],
                                    start=(pat == 0),
                                    stop=(pat == 4),
                                    tile_position=(0, half * 64),
                                )
                            yb = bl["ybase"] + 128 * f
                            ybf = pyo.tile(
                                [128, 128], bf16, tag="ybf",
                                name=f"yb_{layer}_{b2}_{f}",
                            )
                            nc.vector.tensor_copy(out=ybf[:], in_=ps[:])
                            nc.sync.dma_start(
                                out=ydram[yb : yb + 128, :], in_=ybf[:]
                            )
                else:
                    p2 = plan["p2p"][layer]
                    for bl in [] if "nop2" in ablate else p2["blocks"]:
                        b2 = bl["b2"]
                        ntil = 10 * bl["fills"]
                        g2_of = {}
                        ng2done = [0]

                        def issue_group2p(
                            g, b2=b2, bl=bl, ntil=ntil, g2_of=g2_of, layer=layer
                        ):
                            gsz = min(GT2, ntil - g * GT2)
                            ixt = pix.tile(
                                [128, GT2 * 8], i16, tag="ix2",
                                name=f"ix2_{layer}_{b2}_{g}",
                            )
                            nc.sync.dma_start(
                                out=ixt[:, : gsz * 8],
                                in_=g2p[layer][
                                    :,
                                    (bl["tbase"] + g * GT2) * 8 : (
                                        bl["tbase"] + g * GT2 + gsz
                                    )
                                    * 8,
                                ],
                            )
                            gt = pg2.tile(
                                [128, GT2, 128], bf16, tag="g2",
                                name=f"g2_{layer}_{b2}_{g}",
                            )
                            nc.gpsimd.dma_gather(
                                out_ap=gt[:, :gsz, :],
                                in_ap=part[b2 * WIN : (b2 + 1) * WIN, :],
                                idxs_ap=ixt[:, : gsz * 8],
                                num_idxs=gsz * 128,
                                num_idxs_reg=gsz * 128,
                                elem_size=128,
                                single_packet=False,
                                queue_num=next_q(),
                            )
                            g2_of[g] = gt
                            g2_of.pop(g - 3, None)

                        for f in range(bl["fills"]):
                            ps = pps.tile(
                                [128, 128], f32, tag="ps2", bufs=2,
                                name=f"p2_{layer}_{b2}_{f}",
                            )
                            for t in range(10):
                                tloc = 10 * f + t
                                g = tloc // GT2
                                while ng2done[0] <= g:
                                    issue_group2p(ng2done[0])
                                    ng2done[0] += 1
                                lo = tloc - g * GT2
                                half = t // 5
                                pat = t % 5
                                nc.tensor.matmul(
                                    out=ps[half * 64 : half * 64 + 64, :],
                                    lhsT=lhs_t[
                                        :, base2 + pat * 64 : base2 + pat * 64 + 64
                                    ],
                                    rhs=g2_of[g][:, lo, :],
                                    start=(pat == 0),
                                    stop=(pat == 4),
                                    tile_position=(0, half * 64),
                                )
                            ybf = pyo.tile(
                                [128, 128], bf16, tag="ybf",
                                name=f"yb_{layer}_{b2}_{f}",
                            )
                            nc.vector.tensor_copy(out=ybf[:], in_=ps[:])
                            nc.gpsimd.indirect_dma_start(
                                out=ydram[:],
                                out_offset=bass.IndirectOffsetOnAxis(
                                    ap=yp_t[layer][:, bl["fbase"] + f : bl["fbase"] + f + 1],
                                    axis=0,
                                ),
                                in_=ybf[:],
                                in_offset=None,
                            )

                if layer < L - 1 and "noag" not in ablate:
                    nc.gpsimd.collective_compute(
                        "AllGather",
                        mybir.AluOpType.bypass,
                        replica_groups=[list(range(NCORES))],
                        ins=[ydram[:].opt()],
                        outs=[srcs[layer + 1][:].opt()],
                    )

            # ---- epilogue ----
            for kind, pos_t in (
                [] if "noepi" in ablate else (("u", bu_t), ("i", bi_t))
            ):
                for blk in range(NBB):
                    gego = pg2.tile([128, 128], f32, tag="bg", name=f"bg_{kind}_{blk}")
                    nc.gpsimd.indirect_dma_start(
                        out=gego[:],
                        out_offset=None,
                        in_=ego[:],
                        in_offset=bass.IndirectOffsetOnAxis(
                            ap=pos_t[:, blk : blk + 1], axis=0
                        ),
                    )
                    gy = []
                    for li, yt in enumerate(ydst):
                        g = pg2.tile(
                            [128, 128], bf16, tag=f"by{li}",
                            name=f"by{li}_{kind}_{blk}",
                        )
                        nc.gpsimd.indirect_dma_start(
                            out=g[:],
                            out_offset=None,
                            in_=yt[:],
                            in_offset=bass.IndirectOffsetOnAxis(
                                ap=pos_t[:, blk : blk + 1], axis=0
                            ),
                        )
                        gy.append(g)
                    s12 = pyo.tile([128, 128], f32, tag="s12", name=f"s12_{kind}_{blk}")
                    nc.vector.tensor_add(out=s12[:], in0=gy[0][:], in1=gy[1][:])
                    y3f = pyo.tile([128, 128], f32, tag="y3f", name=f"y3f_{kind}_{blk}")
                    nc.vector.tensor_copy(out=y3f[:], in_=gy[2][:])
                    s123 = pyo.tile(
                        [128, 128], f32, tag="s123", name=f"s123_{kind}_{blk}"
                    )
                    nc.vector.tensor_add(out=s123[:], in0=s12[:], in1=y3f[:])
                    allt = pyo.tile([128, 128], f32, tag="allt", name=f"al_{kind}_{blk}")
                    nc.vector.tensor_add(out=allt[:], in0=s123[:], in1=gego[:])
                    tps = pps.tile(
                        [64, 128], f32, tag="tps", bufs=1, name=f"tp_{kind}_{blk}"
                    )
                    nc.tensor.transpose(
                        out=tps[:], in_=allt[:, 0:64], identity=ident_t[:]
                    )
                    xt = pyo.tile([64, 128], f32, tag="xt", name=f"xt_{kind}_{blk}")
                    nc.vector.tensor_copy(out=xt[:], in_=tps[:])
                    pmm = pps.tile(
                        [128, 64], f32, tag="pmm", bufs=1, name=f"pm_{kind}_{blk}"
                    )
                    nc.tensor.matmul(
                        out=pmm[:], lhsT=xt[:], rhs=w4t_t[:], start=True, stop=True
                    )
                    prd = pyo.tile([128, 64], f32, tag="prd", name=f"pd_{kind}_{blk}")
                    nc.vector.tensor_add(out=prd[:], in0=pmm[:], in1=brep_t[:])
                    nc.sync.dma_start(
                        out=outs[f"{kind}_pred"][blk * 128 : (blk + 1) * 128, :],
                        in_=prd[:],
                    )
                    tgt = pyo.tile([128, 64], f32, tag="tgt", name=f"tg_{kind}_{blk}")
                    nc.vector.tensor_scalar_mul(
                        out=tgt[:], in0=allt[:, 64:128], scalar1=0.25
                    )
                    nc.sync.dma_start(
                        out=outs[f"{kind}_target"][blk * 128 : (blk + 1) * 128, :],
                        in_=tgt[:],
                    )

    nc.compile()
    return nc


def _wrap16(arr_128_nt):
    """[128, NT] lane/tile array -> [128, NT*8] int16 dma_gather wrap
    (position i = tile*128 + lane at [i%16, i//16]), replicated 8x."""
    flat = arr_128_nt.T.reshape(-1)
    w = np.ascontiguousarray(flat.reshape(-1, 16).T)
    return np.tile(w, (8, 1))


def _make_in_maps(plan, inputs):
    bf = ml_dtypes.bfloat16
    ego2 = np.concatenate(
        [
            np.concatenate(
                [inputs["user_emb_online"], inputs["item_emb_online"]], axis=0
            ),
            np.concatenate(
                [inputs["user_emb_target"], inputs["item_emb_target"]], axis=0
            ),
        ],
        axis=1,
    ).astype(np.float32)
    x2_0 = np.zeros((NP, 128), bf)
    ego_s = np.zeros((NCORES, S, 128), np.float32)
    for k in range(NCORES):
        x2_0[k * S : k * S + RPC] = ego2[k * RPC : (k + 1) * RPC].astype(bf)
        ego_s[k, :RPC] = ego2[k * RPC : (k + 1) * RPC]

    base1, base2, RSUM = _lhs_layout()
    lhs = np.zeros((128, RSUM), bf)
    for wi, w in enumerate(W1):
        for j in range(R1[wi]):
            lhs[j * w : (j + 1) * w, base1[wi] + j] = 1.0
    for pat in range(5):
        for l in range(128):
            m = (pat * 128 + l) // W2
            lhs[l, base2 + pat * 64 + m % 64] = 1.0

    w4t = np.ascontiguousarray(inputs["pred_w"].astype(np.float32).T * 0.25)
    brep = np.tile(inputs["pred_b"].astype(np.float32)[None, :], (128, 1))
    ident = np.eye(128, dtype=np.float32)

    in_maps = []
    for k in range(NCORES):
        m = {
            "x2_0": x2_0,
            "ego": ego_s[k],
            "g2idx0": _wrap16(plan["gidx2_full"][k]),
            "lhs": lhs,
            "w4t": w4t,
            "brep": brep,
            "ident": ident,
            "bupos": plan["bu_pos"][k],
            "bipos": plan["bi_pos"][k],
        }
        for l in range(L):
            pl = plan["layers"][l]
            m[f"g1idx_{l}"] = _wrap16(pl["gidx1"][k])
            m[f"val1_{l}"] = pl["val1"][k].astype(bf)
        for l in (1, 2):
            p2 = plan["p2p"][l]
            m[f"g2idx_{l}"] = _wrap16(p2["g2"][k])
            m[f"ypos_{l}"] = p2["ypos"][k]
        in_maps.append(m)
    return in_maps


_CACHE = {}
last_exec_ns = None
last_results = None


def kernel(**inputs):
    global last_exec_ns, last_results
    from concourse.bass_utils import run_bass_kernel_spmd

    inputs = {k: np.asarray(v) for k, v in inputs.items()}
    key = (
        inputs["adj_row"].tobytes()[:4096],
        inputs["adj_col"].tobytes()[:4096],
        inputs["user_idx"].tobytes(),
        inputs["item_idx"].tobytes(),
        int(inputs["adj_row"].shape[0]),
    )
    if key in _CACHE:
        plan, nc = _CACHE[key]
    else:
        plan = _plan(
            inputs["adj_row"],
            inputs["adj_col"],
            inputs["adj_val"],
            inputs["user_idx"],
            inputs["item_idx"],
        )
        plan = _plan_batch(plan, inputs["user_idx"], inputs["item_idx"])
        nc = _build_program(plan)
        _CACHE[key] = (plan, nc)

    in_maps = _make_in_maps(plan, inputs)
    global _LAST_IN_MAPS
    _LAST_IN_MAPS = in_maps
    res = run_bass_kernel_spmd(nc, in_maps, list(range(NCORES)))
    last_exec_ns = res.exec_time_ns
    last_results = res

    out = []
    for kind in ("u", "i"):
        owner = plan[f"b{kind}_owner"]
        slot = plan[f"b{kind}_slot"]
        pred = np.empty((B, 64), np.float32)
        tgt = np.empty((B, 64), np.float32)
        for k in range(NCORES):
            m = owner == k
            pred[m] = res.results[k][f"{kind}_pred"][slot[m]]
            tgt[m] = res.results[k][f"{kind}_target"][slot[m]]
        out.append((pred, tgt))
    return (out[0][0], out[0][1], out[1][0], out[1][1])


def measure_time(iters=4):
    """Wall-clock the compiled NEFF on warm device-resident inputs.
    Returns (best_seconds, all_times). Uses the cached plan/nc + last inputs
    (call kernel() first)."""
    import time as _time

    import jax
    from jax.sharding import Mesh, PartitionSpec
    from jax.experimental.shard_map import shard_map
    import concourse.mybir as mybir
    from concourse import bass2jax

    assert _CACHE, "call kernel() first"
    (plan, nc) = next(iter(_CACHE.values()))
    in_maps = _LAST_IN_MAPS
    assert in_maps is not None

    partition_name = nc.partition_id_tensor.name if nc.partition_id_tensor else None
    in_names, out_names, out_avals, zero_outs = [], [], [], []
    for alloc in nc.m.functions[0].allocations:
        if not isinstance(alloc, mybir.MemoryLocationSet):
            continue
        name = alloc.memorylocations[0].name
        if alloc.kind == "ExternalInput":
            if name != partition_name:
                in_names.append(name)
        elif alloc.kind == "ExternalOutput":
            out_names.append(name)
            shape = tuple(alloc.tensor_shape)
            dtype = mybir.dt.np(alloc.dtype)
            out_avals.append(jax.core.ShapedArray(shape, dtype))
            zero_outs.append(np.zeros(shape, dtype))
    n_params = len(in_names)
    n_outs = len(out_avals)
    all_in_names = list(in_names) + out_names
    if partition_name is not None:
        all_in_names.append(partition_name)

    def _body(*args):
        operands = list(args)
        if partition_name is not None:
            operands.append(bass2jax.partition_id_tensor())
        outs = bass2jax._bass_exec_p.bind(
            *operands,
            out_avals=tuple(out_avals),
            in_names=tuple(all_in_names),
            out_names=tuple(out_names),
            lowering_input_output_aliases=(),
            sim_require_finite=True,
            sim_require_nnan=True,
            nc=nc,
        )
        return tuple(outs)

    devices = jax.devices()[:NCORES]
    mesh = Mesh(np.asarray(devices), ("core",))
    donate = tuple(range(n_params, n_params + n_outs))
    sharded = jax.jit(
        shard_map(
            _body,
            mesh=mesh,
            in_specs=(PartitionSpec("core"),) * (n_params + n_outs),
            out_specs=(PartitionSpec("core"),) * n_outs,
            check_rep=False,
        ),
        donate_argnums=donate,
        keep_unused=True,
    )
    from jax.sharding import NamedSharding

    shard = NamedSharding(mesh, PartitionSpec("core"))
    concat_in = [
        jax.device_put(
            np.concatenate([np.asarray(in_maps[c][n]) for c in range(NCORES)], 0),
            shard,
        )
        for n in in_names
    ]
    times = []
    out = None
    for it in range(iters):
        zeros_dev = [
            jax.device_put(
                np.zeros((NCORES * z.shape[0], *z.shape[1:]), z.dtype), shard
            )
            for z in zero_outs
        ]
        jax.block_until_ready(zeros_dev)
        t0 = _time.perf_counter()
        out = sharded(*concat_in, *zeros_dev)
        jax.block_until_ready(out)
        times.append(_time.perf_counter() - t0)
    return min(times[1:]) if len(times) > 1 else times[0], times


_LAST_IN_MAPS = None


# revision 16
# speedup vs baseline: 1.0720x; 1.0720x over previous
"""BUIR (LightGCN x2 + predictor) Trainium2 kernel — 8 NeuronCores, SPMD.

v5 = v3 two-phase SpMM + batch-driven layer pruning:
  - Outputs only need acc = ego+y1+y2+y3 at ~8k batch nodes. Backward
    reachability: layer-3 rows = batch nodes (172k edges, 2.7%), layer-2
    rows = cols(E3)+batch (2.9M edges, 45%), layer-1 full. Each layer
    gets its own phase-1 plan (cells/fills/quads) over its edge subset.
  - Phase-2 for layers 2/3 runs on packed 128-row fills of ACTIVE rows
    only; y rows land via indirect scatter-DMA (positions per fill).
    Layer-1 phase-2 unchanged (dense affine writes).
  - acc tensor + per-fill accumulate-DMAs are gone: the epilogue gathers
    ego/y1/y2/y3 at batch positions, folds them on DVE, then applies the
    predictor. All tables stay [S,128] per core (no packing), so col
    indexing and the AllGathers are identical to v3.
"""

import os
import sys

sys.path.insert(0, "/opt/trn_rl_repo")

import numpy as np
import ml_dtypes

U, NI, D = 100000, 200000, 64
N = U + NI
NNZ = 6400000
B = 4096
L = 3
NCORES = 8
RPC = N // NCORES  # 37500
WIN = 32768
B_ROWS = 2560
NBLK = -(-RPC // B_ROWS)  # 15
W1 = (1, 2, 3, 4, 6, 8, 12, 16)
NW1 = len(W1)
R1 = tuple(128 // w for w in W1)
RSLOT = (128, 64, 64, 32, 32, 32, 32, 32)
PERFILL = (1, 2, 2, 4, 4, 4, 4, 4)
W2 = 10  # phase-2 lanes per row (one per chunk)
GT1 = 48
GT2 = 64

_W1LUT = np.zeros(200, np.int64)
for _d in range(1, 200):
    _W1LUT[_d] = next((i for i, w in enumerate(W1) if _d <= w), NW1 - 1)

# static full phase-2 layout (layer 1 only)
_P2_BLOCKS = []
_S = 0
for _b in range(NBLK):
    _rows = min(B_ROWS, RPC - _b * B_ROWS)
    _fills = -(-_rows // 128)
    _P2_BLOCKS.append(
        dict(rows=_rows, fills=_fills, tiles=10 * _fills, ybase=_S, tbase=0)
    )
    _S += 128 * _fills
_tb = 0
for _bl in _P2_BLOCKS:
    _bl["tbase"] = _tb
    _tb += _bl["tiles"]
S = _S  # 37504
NP = NCORES * S
NCH = -(-NP // WIN)
assert NCH <= W2, (NCH, W2)
NT2 = _tb


# ---------------- planner ----------------


def _plan_phase1(adj_row, adj_col, adj_val, mask):
    """Phase-1 plan for one edge subset. Returns dict with the program
    structure (max envelope across cores) + per-core gidx/val/wpos."""
    adj_row = adj_row[mask]
    adj_col = adj_col[mask]
    adj_val = adj_val[mask]
    core_n = adj_col // RPC
    sig_col = core_n * S + (adj_col - core_n * RPC)
    ch_all = sig_col // WIN
    widx_all = sig_col - ch_all * WIN
    core_all = adj_row // RPC

    percore = []
    for k in range(NCORES):
        m = core_all == k
        lr = adj_row[m] - k * RPC
        ch = ch_all[m]
        wx = widx_all[m]
        v = adj_val[m]
        key = lr * NCH + ch
        srt = np.argsort(key, kind="stable")
        lr, ch, wx, v, key = lr[srt], ch[srt], wx[srt], v[srt], key[srt]
        cnt = np.bincount(key, minlength=RPC * NCH)
        gkey = np.nonzero(cnt)[0]
        gdeg = cnt[gkey]
        if len(gdeg):
            assert gdeg.max() <= W1[-1], gdeg.max()
        g_wi = _W1LUT[gdeg] if len(gdeg) else gdeg
        g_b = (gkey // NCH) // B_ROWS
        cell = (g_b * NW1 + g_wi) * NCH + gkey % NCH
        gsrt = np.argsort(cell, kind="stable")
        percore.append(
            dict(wx=wx, v=v, key=key, cnt=cnt, gkey=gkey, cell=cell, gsrt=gsrt)
        )

    NCELL = NBLK * NW1 * NCH
    cell_tiles = np.zeros(NCELL, np.int64)
    wi_of_cell = (np.arange(NCELL) // NCH) % NW1
    r1_of_cell = np.array(R1)[wi_of_cell]
    for pc in percore:
        c_cnt = np.bincount(pc["cell"], minlength=NCELL)
        cell_tiles = np.maximum(cell_tiles, -(-c_cnt // r1_of_cell))
    cell_tiles = cell_tiles.reshape(NBLK, NW1, NCH)
    for b in range(NBLK):
        for wi in range(NW1):
            pad = (-cell_tiles[b, wi].sum()) % 4
            if pad:
                cell_tiles[b, wi, NCH - 1] += pad
    cell_tiles = cell_tiles.reshape(NCELL)
    cell_t0 = np.concatenate([[0], np.cumsum(cell_tiles)])[:-1]
    NT1 = int(cell_tiles.sum())
    if NT1 == 0:
        NT1 = 4
        cell_tiles[NCH - 1] = 4
        cell_t0 = np.concatenate([[0], np.cumsum(cell_tiles)])[:-1]

    tile_b = np.zeros(NT1, np.int64)
    tile_wi = np.zeros(NT1, np.int64)
    tile_ch = np.zeros(NT1, np.int64)
    for c in range(NCELL):
        t0, nt = cell_t0[c], cell_tiles[c]
        if nt:
            tile_b[t0 : t0 + nt] = c // (NW1 * NCH)
            tile_wi[t0 : t0 + nt] = (c // NCH) % NW1
            tile_ch[t0 : t0 + nt] = c % NCH
    NQ1 = NT1 // 4
    quad_b = tile_b[::4]
    quad_wi = tile_wi[::4]
    quad_R = np.array(R1)[quad_wi]
    assert (tile_wi[::4] == tile_wi[3::4]).all()
    assert (tile_b[::4] == tile_b[3::4]).all()

    fills1 = []  # (q0, nq, b, pbase)
    quad_p0 = np.zeros(NQ1, np.int64)
    quad_fill = np.zeros(NQ1, np.int64)
    win_used = np.ones(NBLK, np.int64)  # slot 0 = zeros
    q = 0
    while q < NQ1:
        b, wi = int(quad_b[q]), int(quad_wi[q])
        nq = 1
        while (
            nq < PERFILL[wi]
            and q + nq < NQ1
            and quad_b[q + nq] == b
            and quad_wi[q + nq] == wi
        ):
            nq += 1
        for i in range(nq):
            quad_p0[q + i] = i * RSLOT[wi]
            quad_fill[q + i] = len(fills1)
        fills1.append((q, nq, b, int(win_used[b])))
        win_used[b] += 512
        q += nq
    assert win_used.max() <= WIN, win_used.max()
    fill_pbase = np.array([f[3] for f in fills1], np.int64)

    quad_of_tile = np.arange(NT1) // 4
    cblk_of_tile = np.arange(NT1) % 4

    gidx1, val1, wpos_l = [], [], []
    for k, pc in enumerate(percore):
        gsrt = pc["gsrt"]
        csort = pc["cell"][gsrt]
        cstart = np.concatenate(
            [[0], np.cumsum(np.bincount(csort, minlength=NCELL))]
        )
        seq = np.arange(len(csort)) - cstart[csort]
        r1c = r1_of_cell[csort]
        g_tile = cell_t0[csort] + seq // r1c
        g_slot = seq % r1c
        assert (seq // r1c < cell_tiles[csort]).all()
        g_w = np.array(W1)[(csort // NCH) % NW1]
        g_lane0 = g_slot * g_w
        gq = quad_of_tile[g_tile]
        gpos = (
            fill_pbase[quad_fill[gq]]
            + cblk_of_tile[g_tile] * 128
            + quad_p0[gq]
            + g_slot
        )
        if len(gpos):
            assert gpos.max() < WIN
        ppos = np.zeros(RPC * NCH, np.int64)
        ppos[pc["gkey"][gsrt]] = gpos
        tile_of_g = np.zeros(RPC * NCH, np.int64)
        lane0_of_g = np.zeros(RPC * NCH, np.int64)
        tile_of_g[pc["gkey"][gsrt]] = g_tile
        lane0_of_g[pc["gkey"][gsrt]] = g_lane0
        key = pc["key"]
        kstart = np.concatenate([[0], np.cumsum(pc["cnt"])])
        rank = np.arange(len(key)) - kstart[key]
        e_tile = tile_of_g[key]
        e_lane = lane0_of_g[key] + rank
        gi = np.zeros((128, NT1), np.int16)
        va = np.zeros((128, NT1), np.float32)
        gi[e_lane, e_tile] = pc["wx"].astype(np.int16)
        va[e_lane, e_tile] = pc["v"]
        gidx1.append(gi)
        val1.append(va)
        wpos_l.append(ppos.reshape(RPC, NCH))

    calls1 = []
    for c in range(NCELL):
        if cell_tiles[c]:
            calls1.append((int(cell_t0[c]), int(cell_tiles[c]), int(c % NCH)))

    return dict(
        NT1=NT1,
        NQ1=NQ1,
        fills1=fills1,
        quad_p0=quad_p0,
        quad_R=quad_R,
        quad_wi=quad_wi,
        calls1=calls1,
        tile_ch=tile_ch,
        gidx1=gidx1,
        val1=val1,
        wpos=wpos_l,
    )


def _plan_phase2_full(pl):
    """Full-coverage phase-2 idx (layer 1): g2idx per core [128, NT2]."""
    gidx2 = []
    t_all = np.arange(NT2)
    blk_of_t = np.zeros(NT2, np.int64)
    for bi, bl in enumerate(_P2_BLOCKS):
        blk_of_t[bl["tbase"] : bl["tbase"] + bl["tiles"]] = bi
    tb_arr = np.array([bl["tbase"] for bl in _P2_BLOCKS])
    rows_b = np.array([bl["rows"] for bl in _P2_BLOCKS])
    for k in range(NCORES):
        wpos = pl["wpos"][k]
        lanes = (
            (t_all - tb_arr[blk_of_t])[None, :] * 128
            + np.arange(128)[:, None]
        )
        row_in_b = lanes // W2
        s_of = lanes % W2
        r_glob = blk_of_t[None, :] * B_ROWS + row_in_b
        valid = (row_in_b < rows_b[blk_of_t][None, :]) & (r_glob < RPC)
        rv = np.where(valid, r_glob, 0)
        g2v = wpos[rv, s_of]
        gidx2.append(np.where(valid, g2v, 0).astype(np.int16))
    return gidx2


def _plan_phase2_packed(pl, act_percore):
    """Packed phase-2 for a pruned layer: fills of 128 ACTIVE rows per
    B_ROWS block; returns program structure + per-core g2 idx and scatter
    positions. Structure (fills per block) is the max envelope over cores."""
    # rows per (core, block)
    rows_cb = [
        [a[(a >= b * B_ROWS) & (a < (b + 1) * B_ROWS)] for b in range(NBLK)]
        for a in act_percore
    ]
    blocks = []  # (b2, fills, tbase, fbase)
    tb = 0
    fb = 0
    for b in range(NBLK):
        mx = max(len(rows_cb[k][b]) for k in range(NCORES))
        fills = -(-mx // 128) if mx else 0
        if fills:
            blocks.append(dict(b2=b, fills=fills, tbase=tb, fbase=fb))
            tb += 10 * fills
            fb += fills
    NT2p = tb
    NF = fb
    g2 = [np.zeros((128, max(NT2p, 1)), np.int16) for _ in range(NCORES)]
    ypos = [
        np.full((128, max(NF, 1)), RPC, np.int32) for _ in range(NCORES)
    ]  # default: dump row
    for k in range(NCORES):
        wpos = pl["wpos"][k]
        for bl in blocks:
            rows = rows_cb[k][bl["b2"]]
            for f in range(bl["fills"]):
                r128 = rows[f * 128 : (f + 1) * 128]
                nr = len(r128)
                if nr == 0:
                    continue
                ypos[k][:nr, bl["fbase"] + f] = r128.astype(np.int32)
                for t in range(10):
                    fl = t * 128 + np.arange(128)
                    ri = (f * 1280 + fl) // 10 - f * 128
                    s = (f * 1280 + fl) % 10
                    ok = ri < nr
                    rv = np.where(ok, r128[np.minimum(ri, max(nr - 1, 0))], 0)
                    gv = wpos[rv, s]
                    g2[k][:, bl["tbase"] + f * 10 + t] = np.where(
                        ok, gv, 0
                    ).astype(np.int16)
    return dict(blocks=blocks, NT2=NT2p, NF=NF, g2=g2, ypos=ypos)


def _plan(adj_row, adj_col, adj_val, user_idx, item_idx):
    adj_row = np.asarray(adj_row, np.int64)
    adj_col = np.asarray(adj_col, np.int64)
    adj_val = np.asarray(adj_val, np.float32)
    batch = np.unique(
        np.concatenate(
            [np.asarray(user_idx, np.int64), U + np.asarray(item_idx, np.int64)]
        )
    )
    m3 = np.zeros(N, bool)
    m3[batch] = True
    e3 = m3[adj_row]
    a2 = m3.copy()
    a2[adj_col[e3]] = True
    e2 = a2[adj_row]
    full = np.ones(NNZ, bool)

    pl0 = _plan_phase1(adj_row, adj_col, adj_val, full)
    pl1 = _plan_phase1(adj_row, adj_col, adj_val, e2)
    pl2 = _plan_phase1(adj_row, adj_col, adj_val, e3)

    gidx2_full = _plan_phase2_full(pl0)

    def act_percore(mask):
        out = []
        nodes = np.nonzero(mask)[0]
        for k in range(NCORES):
            mm = (nodes >= k * RPC) & (nodes < (k + 1) * RPC)
            out.append(np.sort(nodes[mm] - k * RPC))
        return out

    p2_1 = _plan_phase2_packed(pl1, act_percore(a2))
    p2_2 = _plan_phase2_packed(pl2, act_percore(m3))

    return dict(
        layers=[pl0, pl1, pl2],
        gidx2_full=gidx2_full,
        p2p=[None, p2_1, p2_2],
    )


def _plan_batch(plan, user_idx, item_idx):
    out = {}
    counts = []
    for name, nodes in (
        ("u", np.asarray(user_idx, np.int64)),
        ("i", U + np.asarray(item_idx, np.int64)),
    ):
        owner = nodes // RPC
        loc = nodes - owner * RPC
        out[name] = (owner, loc)
        counts.append(np.bincount(owner, minlength=NCORES).max())
    BMAX = int(-(-max(counts) // 128) * 128)
    plan["BMAX"], plan["NBB"] = BMAX, BMAX // 128
    for name in ("u", "i"):
        owner, loc = out[name]
        pos_arr = np.zeros((NCORES, BMAX), np.int64)
        slot = np.zeros(B, np.int64)
        for k in range(NCORES):
            m = np.nonzero(owner == k)[0]
            pos_arr[k, : len(m)] = loc[m]
            slot[m] = np.arange(len(m))
        plan[f"b{name}_pos"] = [
            pos_arr[k].reshape(BMAX // 128, 128).T.astype(np.int32).copy()
            for k in range(NCORES)
        ]
        plan[f"b{name}_owner"] = owner
        plan[f"b{name}_slot"] = slot
    return plan


# ---------------- bass program ----------------


def _lhs_layout():
    base1 = {}
    off = 0
    for wi in range(NW1):
        base1[wi] = off
        off += R1[wi]
    base2 = off  # 5 patterns of 64 cols each
    return base1, base2, off + 5 * 64


def _phase1_segs(pl):
    segs = []
    for t0, nt, ch in pl["calls1"]:
        t = t0
        while t < t0 + nt:
            g = t // GT1
            n = min(t0 + nt, (g + 1) * GT1) - t
            segs.append((g, t - g * GT1, n, ch))
            t += n
    segs.sort(key=lambda s: (s[0], s[1]))
    by_g = {}
    for s in segs:
        by_g.setdefault(s[0], []).append(s)
    return by_g


def _build_program(plan):
    ablate = set(os.environ.get("BUIR_ABLATE", "").split(","))
    import concourse.bass as bass
    import concourse.bacc as bacc
    import concourse.mybir as mybir
    import concourse.tile as tile

    f32, bf16, i16, i32 = (
        mybir.dt.float32,
        mybir.dt.bfloat16,
        mybir.dt.int16,
        mybir.dt.int32,
    )
    NBB = plan["NBB"]
    base1, base2, RSUM = _lhs_layout()
    layers = plan["layers"]
    segs_by_layer = [_phase1_segs(pl) for pl in layers]

    nc = bacc.Bacc(
        "TRN2",
        target_bir_lowering=False,
        debug=False,
        num_devices=NCORES,
        num_swdge_queues=4,
    )
    qctr = [0]

    def next_q():
        qctr[0] = (qctr[0] + 1) % 4
        return qctr[0]
    x2_0 = nc.declare_dram_parameter("x2_0", [NP, 128], bf16, isOutput=False)
    ego = nc.declare_dram_parameter("ego", [S, 128], f32, isOutput=False)
    g1p, v1p = [], []
    for l in range(L):
        nt = layers[l]["NT1"]
        g1p.append(
            nc.declare_dram_parameter(f"g1idx_{l}", [128, nt * 8], i16, isOutput=False)
        )
        v1p.append(
            nc.declare_dram_parameter(f"val1_{l}", [128, nt], bf16, isOutput=False)
        )
    g2idx0 = nc.declare_dram_parameter("g2idx0", [128, NT2 * 8], i16, isOutput=False)
    g2p, ypp = [None], [None]
    for l in (1, 2):
        p2 = plan["p2p"][l]
        g2p.append(
            nc.declare_dram_parameter(
                f"g2idx_{l}", [128, max(p2["NT2"], 1) * 8], i16, isOutput=False
            )
        )
        ypp.append(
            nc.declare_dram_parameter(
                f"ypos_{l}", [128, max(p2["NF"], 1)], i32, isOutput=False
            )
        )
    lhs = nc.declare_dram_parameter("lhs", [128, RSUM], bf16, isOutput=False)
    w4t = nc.declare_dram_parameter("w4t", [64, 64], f32, isOutput=False)
    brep = nc.declare_dram_parameter("brep", [128, 64], f32, isOutput=False)
    ident = nc.declare_dram_parameter("ident", [128, 128], f32, isOutput=False)
    bupos = nc.declare_dram_parameter("bupos", [128, NBB], i32, isOutput=False)
    bipos = nc.declare_dram_parameter("bipos", [128, NBB], i32, isOutput=False)
    outs = {}
    for nm in ("u_pred", "u_target", "i_pred", "i_target"):
        outs[nm] = nc.declare_dram_parameter(nm, [plan["BMAX"], 64], f32, isOutput=True)

    with tile.TileContext(nc) as tc:
        with (
            tc.tile_pool(name="const", bufs=1) as cp,
            tc.tile_pool(name="g1", bufs=4) as pg1,
            tc.tile_pool(name="pr", bufs=3) as ppr,
            tc.tile_pool(name="g2", bufs=3) as pg2,
            tc.tile_pool(name="ix", bufs=4) as pix,
            tc.tile_pool(name="yo", bufs=4) as pyo,
            tc.tile_pool(name="ps", bufs=4, space="PSUM") as pps,
            tc.tile_pool(name="dram", bufs=1, space="DRAM") as dp,
        ):
            val1_t = []
            for l in range(L):
                vt = cp.tile([128, layers[l]["NT1"]], bf16)
                nc.sync.dma_start(out=vt[:], in_=v1p[l][:])
                val1_t.append(vt)
            lhs_t = cp.tile([128, RSUM], bf16)
            w4t_t = cp.tile([64, 64], f32)
            brep_t = cp.tile([128, 64], f32)
            ident_t = cp.tile([128, 128], f32)
            bu_t = cp.tile([128, NBB], i32)
            bi_t = cp.tile([128, NBB], i32)
            yp_t = [None]
            for l in (1, 2):
                t = cp.tile([128, max(plan["p2p"][l]["NF"], 1)], i32)
                nc.sync.dma_start(out=t[:], in_=ypp[l][:])
                yp_t.append(t)
            zrow = cp.tile([1, 128], bf16)
            nc.sync.dma_start(out=lhs_t[:], in_=lhs[:])
            nc.sync.dma_start(out=w4t_t[:], in_=w4t[:])
            nc.sync.dma_start(out=brep_t[:], in_=brep[:])
            nc.sync.dma_start(out=ident_t[:], in_=ident[:])
            nc.sync.dma_start(out=bu_t[:], in_=bupos[:])
            nc.sync.dma_start(out=bi_t[:], in_=bipos[:])
            nc.vector.memzero(zrow[:])

            y1 = dp.tile([S, 128], bf16, name="y1")
            y2 = dp.tile([S, 128], bf16, name="y2")
            y3 = dp.tile([S, 128], bf16, name="y3")
            x2_1 = dp.tile([NP, 128], bf16, addr_space="Shared", name="x2_1")
            x2_2 = dp.tile([NP, 128], bf16, addr_space="Shared", name="x2_2")
            part = dp.tile([NBLK * WIN, 128], bf16, name="part")

            for b in range(NBLK):
                nc.sync.dma_start(out=part[b * WIN : b * WIN + 1, :], in_=zrow[:])

            srcs = [x2_0, x2_1, x2_2]
            ydst = [y1, y2, y3]
            nlayers = 1 if "l1" in ablate else L
            for layer in range(nlayers):
                pl = layers[layer]
                NT1 = pl["NT1"]
                lsrc = srcs[layer]
                segs1_by_group = segs_by_layer[layer]
                qp0 = pl["quad_p0"]
                qR = pl["quad_R"]
                qwi = pl["quad_wi"]
                # ---- phase 1 ----
                prod_of = {}
                nextg = [0]

                def issue_group1(
                    g,
                    lname=f"l{layer}",
                    pl=pl,
                    NT1=NT1,
                    lsrc=lsrc,
                    segs1_by_group=segs1_by_group,
                    prod_of=prod_of,
                    layer=layer,
                ):
                    gsz = min(GT1, NT1 - g * GT1)
                    ixt = pix.tile(
                        [128, GT1 * 8], i16, tag="ix1", name=f"ix1_{lname}_{g}"
                    )
                    nc.sync.dma_start(
                        out=ixt[:, : gsz * 8],
                        in_=g1p[layer][:, g * GT1 * 8 : (g * GT1 + gsz) * 8],
                    )
                    gath = pg1.tile(
                        [128, GT1, 128], bf16, tag="g1", name=f"g1_{lname}_{g}"
                    )
                    for _, lo, nt, ch in segs1_by_group.get(g, []):
                        wrows = min(WIN, NP - ch * WIN)
                        nc.gpsimd.dma_gather(
                            out_ap=gath[:, lo : lo + nt, :],
                            in_ap=lsrc[ch * WIN : ch * WIN + wrows, :],
                            idxs_ap=ixt[:, lo * 8 : (lo + nt) * 8],
                            num_idxs=nt * 128,
                            num_idxs_reg=nt * 128,
                            elem_size=128,
                            single_packet=False,
                            queue_num=next_q(),
                        )
                    prod = ppr.tile(
                        [128, GT1 * 128], bf16, tag="pr", name=f"pr_{lname}_{g}"
                    )
                    nc.vector.tensor_tensor(
                        out=prod[:, : gsz * 128].rearrange(
                            "p (g d) -> p g d", g=gsz
                        ),
                        in0=gath[:, :gsz, :],
                        in1=val1_t[layer][:, g * GT1 : g * GT1 + gsz]
                        .unsqueeze(2)
                        .to_broadcast([128, gsz, 128]),
                        op=mybir.AluOpType.mult,
                    )
                    prod_of[g] = prod
                    prod_of.pop(g - 3, None)

                for fi, (q0, nq, fb, pbase) in enumerate(
                    [] if "nop1" in ablate else pl["fills1"]
                ):
                    ps = pps.tile([128, 512], f32, tag="ps", name=f"p1_{layer}_{fi}")
                    for q in range(q0, q0 + nq):
                        g = (4 * q) // GT1
                        while nextg[0] <= g:
                            issue_group1(nextg[0])
                            nextg[0] += 1
                        lo = 4 * q - g * GT1
                        nc.tensor.matmul(
                            out=ps[int(qp0[q]) : int(qp0[q] + qR[q]), :],
                            lhsT=lhs_t[
                                :, base1[int(qwi[q])] : base1[int(qwi[q])] + int(qR[q])
                            ],
                            rhs=prod_of[g][:, lo * 128 : (lo + 4) * 128],
                            start=True,
                            stop=True,
                            tile_position=(0, int(qp0[q])),
                        )
                    pb = pyo.tile([128, 512], bf16, tag="p1o", name=f"po_{layer}_{fi}")
                    nc.vector.tensor_copy(out=pb[:], in_=ps[:])
                    dst = part[
                        fb * WIN + pbase : fb * WIN + pbase + 512, :
                    ].rearrange("(c p) d -> p c d", c=4)
                    nc.sync.dma_start(
                        out=dst, in_=pb[:].rearrange("p (c d) -> p c d", c=4)
                    )

                # ---- phase 2 ----
                ydram = ydst[layer]
                if layer == 0:
                    for b2, bl in enumerate([] if "nop2" in ablate else _P2_BLOCKS):
                        ntil = bl["tiles"]
                        g2_of = {}
                        ng2done = [0]

                        def issue_group2(g, b2=b2, bl=bl, ntil=ntil, g2_of=g2_of):
                            gsz = min(GT2, ntil - g * GT2)
                            ixt = pix.tile(
                                [128, GT2 * 8], i16, tag="ix2",
                                name=f"ix2_{layer}_{b2}_{g}",
                            )
                            nc.sync.dma_start(
                                out=ixt[:, : gsz * 8],
                                in_=g2idx0[
                                    :,
                                    (bl["tbase"] + g * GT2) * 8 : (
                                        bl["tbase"] + g * GT2 + gsz
                                    )
                                    * 8,
                                ],
                            )
                            gt = pg2.tile(
                                [128, GT2, 128], bf16, tag="g2",
                                name=f"g2_{layer}_{b2}_{g}",
                            )
                            nc.gpsimd.dma_gather(
                                out_ap=gt[:, :gsz, :],
                                in_ap=part[b2 * WIN : (b2 + 1) * WIN, :],
                                idxs_ap=ixt[:, : gsz * 8],
                                num_idxs=gsz * 128,
                                num_idxs_reg=gsz * 128,
                                elem_size=128,
                                single_packet=False,
                                queue_num=next_q(),
                            )
                            g2_of[g] = gt
                            g2_of.pop(g - 3, None)

                        for f in range(bl["fills"]):
                            ps = pps.tile(
                                [128, 128], f32, tag="ps2", bufs=2,
                                name=f"p2_{layer}_{b2}_{f}",
                            )
                            for t in range(10):
                                tloc = 10 * f + t
                                g = tloc // GT2
                                while ng2done[0] <= g:
                                    issue_group2(ng2done[0])
                                    ng2done[0] += 1
                                lo = tloc - g * GT2
                                half = t // 5
                                pat = t % 5
                                nc.tensor.matmul(
                                    out=ps[half * 64 : half * 64 + 64, :],
                                    lhsT=lhs_t[
                                        :, base2 + pat * 64 : base2 + pat * 64 + 64
                                    ],
                                    rhs=g2_of[g][:, lo, :],
                                    start=(pat == 0),
                                    stop=(pat == 4),
                                    tile_position=(0, half * 64),
                                )
                            yb = bl["ybase"] + 128 * f
                            ybf = pyo.tile(
                                [128, 128], bf16, tag="ybf",
                                name=f"yb_{layer}_{b2}_{f}",
                            )
                            nc.vector.tensor_copy(out=ybf[:], in_=ps[:])
                            nc.sync.dma_start(
                                out=ydram[yb : yb + 128, :], in_=ybf[:]
                            )
                else:
                    p2 = plan["p2p"][layer]
                    for bl in [] if "nop2" in ablate else p2["blocks"]:
                        b2 = bl["b2"]
                        ntil = 10 * bl["fills"]
                        g2_of = {}
                        ng2done = [0]

                        def issue_group2p(
                            g, b2=b2, bl=bl, ntil=ntil, g2_of=g2_of, layer=layer
                        ):
                            gsz = min(GT2, ntil - g * GT2)
                            ixt = pix.tile(
                                [128, GT2 * 8], i16, tag="ix2",
                                name=f"ix2_{layer}_{b2}_{g}",
                            )
                            nc.sync.dma_start(
                                out=ixt[:, : gsz * 8],
                                in_=g2p[layer][
                                    :,
                                    (bl["tbase"] + g * GT2) * 8 : (
                                        bl["tbase"] + g * GT2 + gsz
                                    )
                                    * 8,
                                ],
                            )
                            gt = pg2.tile(
                                [128, GT2, 128], bf16, tag="g2",
                                name=f"g2_{layer}_{b2}_{g}",
                            )
                            nc.gpsimd.dma_gather(
                                out_ap=gt[:, :gsz, :],
                                in_ap=part[b2 * WIN : (b2 + 1) * WIN, :],
                                idxs_ap=ixt[:, : gsz * 8],
                                num_idxs=gsz * 128,
                                num_idxs_reg=gsz * 128,
                                elem_size=128,
                                single_packet=False,
                                queue_num=next_q(),
                            )
                            g2_of[g] = gt
                            g2_of.pop(g - 3, None)

                        for f in range(bl["fills"]):
                            ps = pps.tile(
                                [128, 128], f32, tag="ps2", bufs=2,
                                name=f"p2_{layer}_{b2}_{f}",
                            )
                            for t in range(10):
                                tloc = 10 * f + t
                                g = tloc // GT2
                                while ng2done[0] <= g:
                                    issue_group2p(ng2done[0])
                                    ng2done[0] += 1
                                lo = tloc - g * GT2
                                half = t // 5
                                pat = t % 5
                                nc.tensor.matmul(
                                    out=ps[half * 64 : half * 64 + 64, :],
                                    lhsT=lhs_t[
                                        :, base2 + pat * 64 : base2 + pat * 64 + 64
                                    ],
                                    rhs=g2_of[g][:, lo, :],
                                    start=(pat == 0),
                                    stop=(pat == 4),
                                    tile_position=(0, half * 64),
                                )
                            ybf = pyo.tile(
                                [128, 128], bf16, tag="ybf",
                                name=f"yb_{layer}_{b2}_{f}",
                            )
                            nc.vector.tensor_copy(out=ybf[:], in_=ps[:])
                            nc.gpsimd.indirect_dma_start(
                                out=ydram[:],
                                out_offset=bass.IndirectOffsetOnAxis(
                                    ap=yp_t[layer][:, bl["fbase"] + f : bl["fbase"] + f + 1],
                                    axis=0,
                                ),
                                in_=ybf[:],
                                in_offset=None,
                            )

                if layer < L - 1 and "noag" not in ablate:
                    nc.gpsimd.collective_compute(
                        "AllGather",
                        mybir.AluOpType.bypass,
                        replica_groups=[list(range(NCORES))],
                        ins=[ydram[:].opt()],
                        outs=[srcs[layer + 1][:].opt()],
                    )

            # ---- epilogue ----
            for kind, pos_t in (
                [] if "noepi" in ablate else (("u", bu_t), ("i", bi_t))
            ):
                for blk in range(NBB):
                    gego = pg2.tile([128, 128], f32, tag="bg", name=f"bg_{kind}_{blk}")
                    nc.gpsimd.indirect_dma_start(
                        out=gego[:],
                        out_offset=None,
                        in_=ego[:],
                        in_offset=bass.IndirectOffsetOnAxis(
                            ap=pos_t[:, blk : blk + 1], axis=0
                        ),
                    )
                    gy = []
                    for li, yt in enumerate(ydst):
                        g = pg2.tile(
                            [128, 128], bf16, tag=f"by{li}",
                            name=f"by{li}_{kind}_{blk}",
                        )
                        nc.gpsimd.indirect_dma_start(
                            out=g[:],
                            out_offset=None,
                            in_=yt[:],
                            in_offset=bass.IndirectOffsetOnAxis(
                                ap=pos_t[:, blk : blk + 1], axis=0
                            ),
                        )
                        gy.append(g)
                    s12 = pyo.tile([128, 128], f32, tag="s12", name=f"s12_{kind}_{blk}")
                    nc.vector.tensor_add(out=s12[:], in0=gy[0][:], in1=gy[1][:])
                    y3f = pyo.tile([128, 128], f32, tag="y3f", name=f"y3f_{kind}_{blk}")
                    nc.vector.tensor_copy(out=y3f[:], in_=gy[2][:])
                    s123 = pyo.tile(
                        [128, 128], f32, tag="s123", name=f"s123_{kind}_{blk}"
                    )
                    nc.vector.tensor_add(out=s123[:], in0=s12[:], in1=y3f[:])
                    allt = pyo.tile([128, 128], f32, tag="allt", name=f"al_{kind}_{blk}")
                    nc.vector.tensor_add(out=allt[:], in0=s123[:], in1=gego[:])
                    tps = pps.tile(
                        [64, 128], f32, tag="tps", bufs=1, name=f"tp_{kind}_{blk}"
                    )
                    nc.tensor.transpose(
                        out=tps[:], in_=allt[:, 0:64], identity=ident_t[:]
                    )
                    xt = pyo.tile([64, 128], f32, tag="xt", name=f"xt_{kind}_{blk}")
                    nc.vector.tensor_copy(out=xt[:], in_=tps[:])
                    pmm = pps.tile(
                        [128, 64], f32, tag="pmm", bufs=1, name=f"pm_{kind}_{blk}"
                    )
                    nc.tensor.matmul(
                        out=pmm[:], lhsT=xt[:], rhs=w4t_t[:], start=True, stop=True
                    )
                    prd = pyo.tile([128, 64], f32, tag="prd", name=f"pd_{kind}_{blk}")
                    nc.vector.tensor_add(out=prd[:], in0=pmm[:], in1=brep_t[:])
                    nc.sync.dma_start(
                        out=outs[f"{kind}_pred"][blk * 128 : (blk + 1) * 128, :],
                        in_=prd[:],
                    )
                    tgt = pyo.tile([128, 64], f32, tag="tgt", name=f"tg_{kind}_{blk}")
                    nc.vector.tensor_scalar_mul(
                        out=tgt[:], in0=allt[:, 64:128], scalar1=0.25
                    )
                    nc.sync.dma_start(
                        out=outs[f"{kind}_target"][blk * 128 : (blk + 1) * 128, :],
                        in_=tgt[:],
                    )

    nc.compile()
    return nc


def _wrap16(arr_128_nt):
    """[128, NT] lane/tile array -> [128, NT*8] int16 dma_gather wrap
    (position i = tile*128 + lane at [i%16, i//16]), replicated 8x."""
    flat = arr_128_nt.T.reshape(-1)
    w = np.ascontiguousarray(flat.reshape(-1, 16).T)
    return np.tile(w, (8, 1))


def _make_in_maps(plan, inputs):
    bf = ml_dtypes.bfloat16
    ego2 = np.concatenate(
        [
            np.concatenate(
                [inputs["user_emb_online"], inputs["item_emb_online"]], axis=0
            ),
            np.concatenate(
                [inputs["user_emb_target"], inputs["item_emb_target"]], axis=0
            ),
        ],
        axis=1,
    ).astype(np.float32)
    x2_0 = np.zeros((NP, 128), bf)
    ego_s = np.zeros((NCORES, S, 128), np.float32)
    for k in range(NCORES):
        x2_0[k * S : k * S + RPC] = ego2[k * RPC : (k + 1) * RPC].astype(bf)
        ego_s[k, :RPC] = ego2[k * RPC : (k + 1) * RPC]

    base1, base2, RSUM = _lhs_layout()
    lhs = np.zeros((128, RSUM), bf)
    for wi, w in enumerate(W1):
        for j in range(R1[wi]):
            lhs[j * w : (j + 1) * w, base1[wi] + j] = 1.0
    for pat in range(5):
        for l in range(128):
            m = (pat * 128 + l) // W2
            lhs[l, base2 + pat * 64 + m % 64] = 1.0

    w4t = np.ascontiguousarray(inputs["pred_w"].astype(np.float32).T * 0.25)
    brep = np.tile(inputs["pred_b"].astype(np.float32)[None, :], (128, 1))
    ident = np.eye(128, dtype=np.float32)

    in_maps = []
    for k in range(NCORES):
        m = {
            "x2_0": x2_0,
            "ego": ego_s[k],
            "g2idx0": _wrap16(plan["gidx2_full"][k]),
            "lhs": lhs,
            "w4t": w4t,
            "brep": brep,
            "ident": ident,
            "bupos": plan["bu_pos"][k],
            "bipos": plan["bi_pos"][k],
        }
        for l in range(L):
            pl = plan["layers"][l]
            m[f"g1idx_{l}"] = _wrap16(pl["gidx1"][k])
            m[f"val1_{l}"] = pl["val1"][k].astype(bf)
        for l in (1, 2):
            p2 = plan["p2p"][l]
            m[f"g2idx_{l}"] = _wrap16(p2["g2"][k])
            m[f"ypos_{l}"] = p2["ypos"][k]
        in_maps.append(m)
    return in_maps


_CACHE = {}
last_exec_ns = None
last_results = None


def kernel(**inputs):
    global last_exec_ns, last_results
    from concourse.bass_utils import run_bass_kernel_spmd

    inputs = {k: np.asarray(v) for k, v in inputs.items()}
    key = (
        inputs["adj_row"].tobytes()[:4096],
        inputs["adj_col"].tobytes()[:4096],
        inputs["user_idx"].tobytes(),
        inputs["item_idx"].tobytes(),
        int(inputs["adj_row"].shape[0]),
    )
    if key in _CACHE:
        plan, nc = _CACHE[key]
    else:
        plan = _plan(
            inputs["adj_row"],
            inputs["adj_col"],
            inputs["adj_val"],
            inputs["user_idx"],
            inputs["item_idx"],
        )
        plan = _plan_batch(plan, inputs["user_idx"], inputs["item_idx"])
        nc = _build_program(plan)
        _CACHE[key] = (plan, nc)

    in_maps = _make_in_maps(plan, inputs)
    global _LAST_IN_MAPS
    _LAST_IN_MAPS = in_maps
    res = run_bass_kernel_spmd(nc, in_maps, list(range(NCORES)))
    last_exec_ns = res.exec_time_ns
    last_results = res

    out = []
    for kind in ("u", "i"):
        owner = plan[f"b{kind}_owner"]
        slot = plan[f"b{kind}_slot"]
        pred = np.empty((B, 64), np.float32)
        tgt = np.empty((B, 64), np.float32)
        for k in range(NCORES):
            m = owner == k
            pred[m] = res.results[k][f"{kind}_pred"][slot[m]]
            tgt[m] = res.results[k][f"{kind}_target"][slot[m]]
        out.append((pred, tgt))
    return (out[0][0], out[0][1], out[1][0], out[1][1])


def measure_time(iters=4):
    """Wall-clock the compiled NEFF on warm device-resident inputs.
    Returns (best_seconds, all_times). Uses the cached plan/nc + last inputs
    (call kernel() first)."""
    import time as _time

    import jax
    from jax.sharding import Mesh, PartitionSpec
    from jax.experimental.shard_map import shard_map
    import concourse.mybir as mybir
    from concourse import bass2jax

    assert _CACHE, "call kernel() first"
    (plan, nc) = next(iter(_CACHE.values()))
    in_maps = _LAST_IN_MAPS
    assert in_maps is not None

    partition_name = nc.partition_id_tensor.name if nc.partition_id_tensor else None
    in_names, out_names, out_avals, zero_outs = [], [], [], []
    for alloc in nc.m.functions[0].allocations:
        if not isinstance(alloc, mybir.MemoryLocationSet):
            continue
        name = alloc.memorylocations[0].name
        if alloc.kind == "ExternalInput":
            if name != partition_name:
                in_names.append(name)
        elif alloc.kind == "ExternalOutput":
            out_names.append(name)
            shape = tuple(alloc.tensor_shape)
            dtype = mybir.dt.np(alloc.dtype)
            out_avals.append(jax.core.ShapedArray(shape, dtype))
            zero_outs.append(np.zeros(shape, dtype))
    n_params = len(in_names)
    n_outs = len(out_avals)
    all_in_names = list(in_names) + out_names
    if partition_name is not None:
        all_in_names.append(partition_name)

    def _body(*args):
        operands = list(args)
        if partition_name is not None:
            operands.append(bass2jax.partition_id_tensor())
        outs = bass2jax._bass_exec_p.bind(
            *operands,
            out_avals=tuple(out_avals),
            in_names=tuple(all_in_names),
            out_names=tuple(out_names),
            lowering_input_output_aliases=(),
            sim_require_finite=True,
            sim_require_nnan=True,
            nc=nc,
        )
        return tuple(outs)

    devices = jax.devices()[:NCORES]
    mesh = Mesh(np.asarray(devices), ("core",))
    donate = tuple(range(n_params, n_params + n_outs))
    sharded = jax.jit(
        shard_map(
            _body,
            mesh=mesh,
            in_specs=(PartitionSpec("core"),) * (n_params + n_outs),
            out_specs=(PartitionSpec("core"),) * n_outs,
            check_rep=False,
        ),
        donate_argnums=donate,
        keep_unused=True,
    )
    from jax.sharding import NamedSharding

    shard = NamedSharding(mesh, PartitionSpec("core"))
    concat_in = [
        jax.device_put(
            np.concatenate([np.asarray(in_maps[c][n]) for c in range(NCORES)], 0),
            shard,
        )
        for n in in_names
    ]
    times = []
    out = None
    for it in range(iters):
        zeros_dev = [
            jax.device_put(
                np.zeros((NCORES * z.shape[0], *z.shape[1:]), z.dtype), shard
            )
            for z in zero_outs
        ]
        jax.block_until_ready(zeros_dev)
        t0 = _time.perf_counter()
        out = sharded(*concat_in, *zeros_dev)
        jax.block_until_ready(out)
        times.append(_time.perf_counter() - t0)
    return min(times[1:]) if len(times) > 1 else times[0], times


_LAST_IN_MAPS = None


# revision 18
# speedup vs baseline: 1.1148x; 1.0399x over previous
"""BUIR (LightGCN x2 + predictor) Trainium2 kernel — 8 NeuronCores, SPMD.

v5 = v3 two-phase SpMM + batch-driven layer pruning:
  - Outputs only need acc = ego+y1+y2+y3 at ~8k batch nodes. Backward
    reachability: layer-3 rows = batch nodes (172k edges, 2.7%), layer-2
    rows = cols(E3)+batch (2.9M edges, 45%), layer-1 full. Each layer
    gets its own phase-1 plan (cells/fills/quads) over its edge subset.
  - Phase-2 for layers 2/3 runs on packed 128-row fills of ACTIVE rows
    only; y rows land via indirect scatter-DMA (positions per fill).
    Layer-1 phase-2 unchanged (dense affine writes).
  - acc tensor + per-fill accumulate-DMAs are gone: the epilogue gathers
    ego/y1/y2/y3 at batch positions, folds them on DVE, then applies the
    predictor. All tables stay [S,128] per core (no packing), so col
    indexing and the AllGathers are identical to v3.
"""

import os
import sys

sys.path.insert(0, "/opt/trn_rl_repo")

import numpy as np
import ml_dtypes

U, NI, D = 100000, 200000, 64
N = U + NI
NNZ = 6400000
B = 4096
L = 3
NCORES = 8
RPC = N // NCORES  # 37500
WIN = 32768
B_ROWS = 2560
NBLK = -(-RPC // B_ROWS)  # 15
W1 = (1, 2, 3, 4, 6, 8, 12, 16)
NW1 = len(W1)
R1 = tuple(128 // w for w in W1)
RSLOT = (128, 64, 64, 32, 32, 32, 32, 32)
PERFILL = (1, 2, 2, 4, 4, 4, 4, 4)
W2 = 10  # phase-2 lanes per row (one per chunk)
GT1 = 48
GT2 = 64

_W1LUT = np.zeros(200, np.int64)
for _d in range(1, 200):
    _W1LUT[_d] = next((i for i, w in enumerate(W1) if _d <= w), NW1 - 1)

# static full phase-2 layout (layer 1 only)
_P2_BLOCKS = []
_S = 0
for _b in range(NBLK):
    _rows = min(B_ROWS, RPC - _b * B_ROWS)
    _fills = -(-_rows // 128)
    _P2_BLOCKS.append(
        dict(rows=_rows, fills=_fills, tiles=10 * _fills, ybase=_S, tbase=0)
    )
    _S += 128 * _fills
_tb = 0
for _bl in _P2_BLOCKS:
    _bl["tbase"] = _tb
    _tb += _bl["tiles"]
S = _S  # 37504
NP = NCORES * S
NCH = -(-NP // WIN)
assert NCH <= W2, (NCH, W2)
NT2 = _tb


# ---------------- planner ----------------


def _plan_phase1(adj_row, adj_col, adj_val, mask, b_rows=B_ROWS, nblk=NBLK):
    """Phase-1 plan for one edge subset. Returns dict with the program
    structure (max envelope across cores) + per-core gidx/val/wpos.
    b_rows/nblk set the partial-window block granularity (coarser blocks
    for sparse layers = fewer cells, less padding, fewer gather calls)."""
    adj_row = adj_row[mask]
    adj_col = adj_col[mask]
    adj_val = adj_val[mask]
    core_n = adj_col // RPC
    sig_col = core_n * S + (adj_col - core_n * RPC)
    ch_all = sig_col // WIN
    widx_all = sig_col - ch_all * WIN
    core_all = adj_row // RPC

    percore = []
    for k in range(NCORES):
        m = core_all == k
        lr = adj_row[m] - k * RPC
        ch = ch_all[m]
        wx = widx_all[m]
        v = adj_val[m]
        key = lr * NCH + ch
        srt = np.argsort(key, kind="stable")
        lr, ch, wx, v, key = lr[srt], ch[srt], wx[srt], v[srt], key[srt]
        cnt = np.bincount(key, minlength=RPC * NCH)
        gkey = np.nonzero(cnt)[0]
        gdeg = cnt[gkey]
        if len(gdeg):
            assert gdeg.max() <= W1[-1], gdeg.max()
        g_wi = _W1LUT[gdeg] if len(gdeg) else gdeg
        g_b = (gkey // NCH) // b_rows
        cell = (g_b * NW1 + g_wi) * NCH + gkey % NCH
        gsrt = np.argsort(cell, kind="stable")
        percore.append(
            dict(wx=wx, v=v, key=key, cnt=cnt, gkey=gkey, cell=cell, gsrt=gsrt)
        )

    NCELL = nblk * NW1 * NCH
    cell_tiles = np.zeros(NCELL, np.int64)
    wi_of_cell = (np.arange(NCELL) // NCH) % NW1
    r1_of_cell = np.array(R1)[wi_of_cell]
    for pc in percore:
        c_cnt = np.bincount(pc["cell"], minlength=NCELL)
        cell_tiles = np.maximum(cell_tiles, -(-c_cnt // r1_of_cell))
    cell_tiles = cell_tiles.reshape(nblk, NW1, NCH)
    for b in range(nblk):
        for wi in range(NW1):
            pad = (-cell_tiles[b, wi].sum()) % 4
            if pad:
                cell_tiles[b, wi, NCH - 1] += pad
    cell_tiles = cell_tiles.reshape(NCELL)
    cell_t0 = np.concatenate([[0], np.cumsum(cell_tiles)])[:-1]
    NT1 = int(cell_tiles.sum())
    if NT1 == 0:
        NT1 = 4
        cell_tiles[NCH - 1] = 4
        cell_t0 = np.concatenate([[0], np.cumsum(cell_tiles)])[:-1]

    tile_b = np.zeros(NT1, np.int64)
    tile_wi = np.zeros(NT1, np.int64)
    tile_ch = np.zeros(NT1, np.int64)
    for c in range(NCELL):
        t0, nt = cell_t0[c], cell_tiles[c]
        if nt:
            tile_b[t0 : t0 + nt] = c // (NW1 * NCH)
            tile_wi[t0 : t0 + nt] = (c // NCH) % NW1
            tile_ch[t0 : t0 + nt] = c % NCH
    NQ1 = NT1 // 4
    quad_b = tile_b[::4]
    quad_wi = tile_wi[::4]
    quad_R = np.array(R1)[quad_wi]
    assert (tile_wi[::4] == tile_wi[3::4]).all()
    assert (tile_b[::4] == tile_b[3::4]).all()

    fills1 = []  # (q0, nq, b, pbase)
    quad_p0 = np.zeros(NQ1, np.int64)
    quad_fill = np.zeros(NQ1, np.int64)
    win_used = np.ones(nblk, np.int64)  # slot 0 = zeros
    q = 0
    while q < NQ1:
        b, wi = int(quad_b[q]), int(quad_wi[q])
        nq = 1
        while (
            nq < PERFILL[wi]
            and q + nq < NQ1
            and quad_b[q + nq] == b
            and quad_wi[q + nq] == wi
        ):
            nq += 1
        for i in range(nq):
            quad_p0[q + i] = i * RSLOT[wi]
            quad_fill[q + i] = len(fills1)
        fills1.append((q, nq, b, int(win_used[b])))
        win_used[b] += 512
        q += nq
    assert win_used.max() <= WIN, win_used.max()
    fill_pbase = np.array([f[3] for f in fills1], np.int64)

    quad_of_tile = np.arange(NT1) // 4
    cblk_of_tile = np.arange(NT1) % 4

    gidx1, val1, wpos_l = [], [], []
    for k, pc in enumerate(percore):
        gsrt = pc["gsrt"]
        csort = pc["cell"][gsrt]
        cstart = np.concatenate(
            [[0], np.cumsum(np.bincount(csort, minlength=NCELL))]
        )
        seq = np.arange(len(csort)) - cstart[csort]
        r1c = r1_of_cell[csort]
        g_tile = cell_t0[csort] + seq // r1c
        g_slot = seq % r1c
        assert (seq // r1c < cell_tiles[csort]).all()
        g_w = np.array(W1)[(csort // NCH) % NW1]
        g_lane0 = g_slot * g_w
        gq = quad_of_tile[g_tile]
        gpos = (
            fill_pbase[quad_fill[gq]]
            + cblk_of_tile[g_tile] * 128
            + quad_p0[gq]
            + g_slot
        )
        if len(gpos):
            assert gpos.max() < WIN
        ppos = np.zeros(RPC * NCH, np.int64)
        ppos[pc["gkey"][gsrt]] = gpos
        tile_of_g = np.zeros(RPC * NCH, np.int64)
        lane0_of_g = np.zeros(RPC * NCH, np.int64)
        tile_of_g[pc["gkey"][gsrt]] = g_tile
        lane0_of_g[pc["gkey"][gsrt]] = g_lane0
        key = pc["key"]
        kstart = np.concatenate([[0], np.cumsum(pc["cnt"])])
        rank = np.arange(len(key)) - kstart[key]
        e_tile = tile_of_g[key]
        e_lane = lane0_of_g[key] + rank
        gi = np.zeros((128, NT1), np.int16)
        va = np.zeros((128, NT1), np.float32)
        gi[e_lane, e_tile] = pc["wx"].astype(np.int16)
        va[e_lane, e_tile] = pc["v"]
        gidx1.append(gi)
        val1.append(va)
        wpos_l.append(ppos.reshape(RPC, NCH))

    calls1 = []
    for c in range(NCELL):
        if cell_tiles[c]:
            calls1.append((int(cell_t0[c]), int(cell_tiles[c]), int(c % NCH)))

    return dict(
        NT1=NT1,
        NQ1=NQ1,
        b_rows=b_rows,
        nblk=nblk,
        fills1=fills1,
        quad_p0=quad_p0,
        quad_R=quad_R,
        quad_wi=quad_wi,
        calls1=calls1,
        tile_ch=tile_ch,
        gidx1=gidx1,
        val1=val1,
        wpos=wpos_l,
    )


def _plan_phase2_full(pl):
    """Full-coverage phase-2 idx (layer 1): g2idx per core [128, NT2]."""
    gidx2 = []
    t_all = np.arange(NT2)
    blk_of_t = np.zeros(NT2, np.int64)
    for bi, bl in enumerate(_P2_BLOCKS):
        blk_of_t[bl["tbase"] : bl["tbase"] + bl["tiles"]] = bi
    tb_arr = np.array([bl["tbase"] for bl in _P2_BLOCKS])
    rows_b = np.array([bl["rows"] for bl in _P2_BLOCKS])
    for k in range(NCORES):
        wpos = pl["wpos"][k]
        lanes = (
            (t_all - tb_arr[blk_of_t])[None, :] * 128
            + np.arange(128)[:, None]
        )
        row_in_b = lanes // W2
        s_of = lanes % W2
        r_glob = blk_of_t[None, :] * B_ROWS + row_in_b
        valid = (row_in_b < rows_b[blk_of_t][None, :]) & (r_glob < RPC)
        rv = np.where(valid, r_glob, 0)
        g2v = wpos[rv, s_of]
        gidx2.append(np.where(valid, g2v, 0).astype(np.int16))
    return gidx2


def _plan_phase2_packed(pl, act_percore):
    """Packed phase-2 for a pruned layer: fills of 128 ACTIVE rows per
    B_ROWS block; returns program structure + per-core g2 idx and scatter
    positions. Structure (fills per block) is the max envelope over cores."""
    # rows per (core, block) at the layer's block granularity
    b_rows, nblk = pl["b_rows"], pl["nblk"]
    rows_cb = [
        [a[(a >= b * b_rows) & (a < (b + 1) * b_rows)] for b in range(nblk)]
        for a in act_percore
    ]
    blocks = []  # (b2, fills, tbase, fbase)
    tb = 0
    fb = 0
    for b in range(nblk):
        mx = max(len(rows_cb[k][b]) for k in range(NCORES))
        fills = -(-mx // 128) if mx else 0
        if fills:
            blocks.append(dict(b2=b, fills=fills, tbase=tb, fbase=fb))
            tb += 10 * fills
            fb += fills
    NT2p = tb
    NF = fb
    g2 = [np.zeros((128, max(NT2p, 1)), np.int16) for _ in range(NCORES)]
    ypos = [
        np.full((128, max(NF, 1)), RPC, np.int32) for _ in range(NCORES)
    ]  # default: dump row
    for k in range(NCORES):
        wpos = pl["wpos"][k]
        for bl in blocks:
            rows = rows_cb[k][bl["b2"]]
            for f in range(bl["fills"]):
                r128 = rows[f * 128 : (f + 1) * 128]
                nr = len(r128)
                if nr == 0:
                    continue
                ypos[k][:nr, bl["fbase"] + f] = r128.astype(np.int32)
                for t in range(10):
                    fl = t * 128 + np.arange(128)
                    ri = (f * 1280 + fl) // 10 - f * 128
                    s = (f * 1280 + fl) % 10
                    ok = ri < nr
                    rv = np.where(ok, r128[np.minimum(ri, max(nr - 1, 0))], 0)
                    gv = wpos[rv, s]
                    g2[k][:, bl["tbase"] + f * 10 + t] = np.where(
                        ok, gv, 0
                    ).astype(np.int16)
    return dict(blocks=blocks, NT2=NT2p, NF=NF, g2=g2, ypos=ypos)


def _plan(adj_row, adj_col, adj_val, user_idx, item_idx):
    adj_row = np.asarray(adj_row, np.int64)
    adj_col = np.asarray(adj_col, np.int64)
    adj_val = np.asarray(adj_val, np.float32)
    batch = np.unique(
        np.concatenate(
            [np.asarray(user_idx, np.int64), U + np.asarray(item_idx, np.int64)]
        )
    )
    m3 = np.zeros(N, bool)
    m3[batch] = True
    e3 = m3[adj_row]
    a2 = m3.copy()
    a2[adj_col[e3]] = True
    e2 = a2[adj_row]
    full = np.ones(NNZ, bool)

    pl0 = _plan_phase1(adj_row, adj_col, adj_val, full)

    def plan_coarse(mask, b_rows, nblk):
        try:
            return _plan_phase1(adj_row, adj_col, adj_val, mask, b_rows, nblk)
        except AssertionError:
            # partial-window capacity overflow at coarse granularity
            return _plan_phase1(adj_row, adj_col, adj_val, mask)

    pl1 = plan_coarse(e2, 3750, 10)
    pl2 = plan_coarse(e3, 12500, 3)

    gidx2_full = _plan_phase2_full(pl0)

    def act_percore(mask):
        out = []
        nodes = np.nonzero(mask)[0]
        for k in range(NCORES):
            mm = (nodes >= k * RPC) & (nodes < (k + 1) * RPC)
            out.append(np.sort(nodes[mm] - k * RPC))
        return out

    p2_1 = _plan_phase2_packed(pl1, act_percore(a2))
    p2_2 = _plan_phase2_packed(pl2, act_percore(m3))

    return dict(
        layers=[pl0, pl1, pl2],
        gidx2_full=gidx2_full,
        p2p=[None, p2_1, p2_2],
    )


def _plan_batch(plan, user_idx, item_idx):
    out = {}
    counts = []
    for name, nodes in (
        ("u", np.asarray(user_idx, np.int64)),
        ("i", U + np.asarray(item_idx, np.int64)),
    ):
        owner = nodes // RPC
        loc = nodes - owner * RPC
        out[name] = (owner, loc)
        counts.append(np.bincount(owner, minlength=NCORES).max())
    BMAX = int(-(-max(counts) // 128) * 128)
    plan["BMAX"], plan["NBB"] = BMAX, BMAX // 128
    for name in ("u", "i"):
        owner, loc = out[name]
        pos_arr = np.zeros((NCORES, BMAX), np.int64)
        slot = np.zeros(B, np.int64)
        for k in range(NCORES):
            m = np.nonzero(owner == k)[0]
            pos_arr[k, : len(m)] = loc[m]
            slot[m] = np.arange(len(m))
        plan[f"b{name}_pos"] = [
            pos_arr[k].reshape(BMAX // 128, 128).T.astype(np.int32).copy()
            for k in range(NCORES)
        ]
        plan[f"b{name}_owner"] = owner
        plan[f"b{name}_slot"] = slot
    return plan


# ---------------- bass program ----------------


def _lhs_layout():
    base1 = {}
    off = 0
    for wi in range(NW1):
        base1[wi] = off
        off += R1[wi]
    base2 = off  # 5 patterns of 64 cols each
    return base1, base2, off + 5 * 64


def _phase1_segs(pl):
    segs = []
    for t0, nt, ch in pl["calls1"]:
        t = t0
        while t < t0 + nt:
            g = t // GT1
            n = min(t0 + nt, (g + 1) * GT1) - t
            segs.append((g, t - g * GT1, n, ch))
            t += n
    segs.sort(key=lambda s: (s[0], s[1]))
    by_g = {}
    for s in segs:
        by_g.setdefault(s[0], []).append(s)
    return by_g


def _build_program(plan):
    ablate = set(os.environ.get("BUIR_ABLATE", "").split(","))
    import concourse.bass as bass
    import concourse.bacc as bacc
    import concourse.mybir as mybir
    import concourse.tile as tile

    f32, bf16, i16, i32 = (
        mybir.dt.float32,
        mybir.dt.bfloat16,
        mybir.dt.int16,
        mybir.dt.int32,
    )
    NBB = plan["NBB"]
    base1, base2, RSUM = _lhs_layout()
    layers = plan["layers"]
    segs_by_layer = [_phase1_segs(pl) for pl in layers]

    nc = bacc.Bacc(
        "TRN2",
        target_bir_lowering=False,
        debug=False,
        num_devices=NCORES,
        num_swdge_queues=4,
    )
    qload = [0, 0, 0, 0]

    def next_q(ndesc=128):
        q = min(range(4), key=lambda i: qload[i])
        qload[q] += ndesc
        return q
    x2_0 = nc.declare_dram_parameter("x2_0", [NP, 128], bf16, isOutput=False)
    ego = nc.declare_dram_parameter("ego", [S, 128], f32, isOutput=False)
    g1p, v1p = [], []
    for l in range(L):
        nt = layers[l]["NT1"]
        g1p.append(
            nc.declare_dram_parameter(f"g1idx_{l}", [128, nt * 8], i16, isOutput=False)
        )
        v1p.append(
            nc.declare_dram_parameter(f"val1_{l}", [128, nt], bf16, isOutput=False)
        )
    g2idx0 = nc.declare_dram_parameter("g2idx0", [128, NT2 * 8], i16, isOutput=False)
    g2p, ypp = [None], [None]
    for l in (1, 2):
        p2 = plan["p2p"][l]
        g2p.append(
            nc.declare_dram_parameter(
                f"g2idx_{l}", [128, max(p2["NT2"], 1) * 8], i16, isOutput=False
            )
        )
        ypp.append(
            nc.declare_dram_parameter(
                f"ypos_{l}", [128, max(p2["NF"], 1)], i32, isOutput=False
            )
        )
    lhs = nc.declare_dram_parameter("lhs", [128, RSUM], bf16, isOutput=False)
    w4t = nc.declare_dram_parameter("w4t", [64, 64], f32, isOutput=False)
    brep = nc.declare_dram_parameter("brep", [128, 64], f32, isOutput=False)
    ident = nc.declare_dram_parameter("ident", [128, 128], f32, isOutput=False)
    bupos = nc.declare_dram_parameter("bupos", [128, NBB], i32, isOutput=False)
    bipos = nc.declare_dram_parameter("bipos", [128, NBB], i32, isOutput=False)
    outs = {}
    for nm in ("u_pred", "u_target", "i_pred", "i_target"):
        outs[nm] = nc.declare_dram_parameter(nm, [plan["BMAX"], 64], f32, isOutput=True)

    with tile.TileContext(nc) as tc:
        with (
            tc.tile_pool(name="const", bufs=1) as cp,
            tc.tile_pool(name="g1", bufs=4) as pg1,
            tc.tile_pool(name="pr", bufs=3) as ppr,
            tc.tile_pool(name="g2", bufs=3) as pg2,
            tc.tile_pool(name="ix", bufs=4) as pix,
            tc.tile_pool(name="yo", bufs=4) as pyo,
            tc.tile_pool(name="ps", bufs=4, space="PSUM") as pps,
            tc.tile_pool(name="dram", bufs=1, space="DRAM") as dp,
        ):
            val1_t = []
            for l in range(L):
                vt = cp.tile([128, layers[l]["NT1"]], bf16)
                nc.sync.dma_start(out=vt[:], in_=v1p[l][:])
                val1_t.append(vt)
            lhs_t = cp.tile([128, RSUM], bf16)
            w4t_t = cp.tile([64, 64], f32)
            brep_t = cp.tile([128, 64], f32)
            ident_t = cp.tile([128, 128], f32)
            bu_t = cp.tile([128, NBB], i32)
            bi_t = cp.tile([128, NBB], i32)
            yp_t = [None]
            for l in (1, 2):
                t = cp.tile([128, max(plan["p2p"][l]["NF"], 1)], i32)
                nc.sync.dma_start(out=t[:], in_=ypp[l][:])
                yp_t.append(t)
            zrow = cp.tile([1, 128], bf16)
            nc.sync.dma_start(out=lhs_t[:], in_=lhs[:])
            nc.sync.dma_start(out=w4t_t[:], in_=w4t[:])
            nc.sync.dma_start(out=brep_t[:], in_=brep[:])
            nc.sync.dma_start(out=ident_t[:], in_=ident[:])
            nc.sync.dma_start(out=bu_t[:], in_=bupos[:])
            nc.sync.dma_start(out=bi_t[:], in_=bipos[:])
            nc.vector.memzero(zrow[:])

            y1 = dp.tile([S, 128], bf16, name="y1")
            y2 = dp.tile([S, 128], bf16, name="y2")
            y3 = dp.tile([S, 128], bf16, name="y3")
            x2_1 = dp.tile([NP, 128], bf16, addr_space="Shared", name="x2_1")
            x2_2 = dp.tile([NP, 128], bf16, addr_space="Shared", name="x2_2")
            part = dp.tile([NBLK * WIN, 128], bf16, name="part")

            for b in range(NBLK):
                nc.sync.dma_start(out=part[b * WIN : b * WIN + 1, :], in_=zrow[:])

            srcs = [x2_0, x2_1, x2_2]
            ydst = [y1, y2, y3]
            nlayers = 1 if "l1" in ablate else L
            for layer in range(nlayers):
                pl = layers[layer]
                NT1 = pl["NT1"]
                lsrc = srcs[layer]
                segs1_by_group = segs_by_layer[layer]
                qp0 = pl["quad_p0"]
                qR = pl["quad_R"]
                qwi = pl["quad_wi"]
                # ---- phase 1 ----
                prod_of = {}
                nextg = [0]

                def issue_group1(
                    g,
                    lname=f"l{layer}",
                    pl=pl,
                    NT1=NT1,
                    lsrc=lsrc,
                    segs1_by_group=segs1_by_group,
                    prod_of=prod_of,
                    layer=layer,
                ):
                    gsz = min(GT1, NT1 - g * GT1)
                    ixt = pix.tile(
                        [128, GT1 * 8], i16, tag="ix1", name=f"ix1_{lname}_{g}"
                    )
                    nc.sync.dma_start(
                        out=ixt[:, : gsz * 8],
                        in_=g1p[layer][:, g * GT1 * 8 : (g * GT1 + gsz) * 8],
                    )
                    gath = pg1.tile(
                        [128, GT1, 128], bf16, tag="g1", name=f"g1_{lname}_{g}"
                    )
                    for _, lo, nt, ch in segs1_by_group.get(g, []):
                        wrows = min(WIN, NP - ch * WIN)
                        nc.gpsimd.dma_gather(
                            out_ap=gath[:, lo : lo + nt, :],
                            in_ap=lsrc[ch * WIN : ch * WIN + wrows, :],
                            idxs_ap=ixt[:, lo * 8 : (lo + nt) * 8],
                            num_idxs=nt * 128,
                            num_idxs_reg=nt * 128,
                            elem_size=128,
                            single_packet=False,
                            queue_num=next_q(nt * 128),
                        )
                    prod = ppr.tile(
                        [128, GT1 * 128], bf16, tag="pr", name=f"pr_{lname}_{g}"
                    )
                    nc.vector.tensor_tensor(
                        out=prod[:, : gsz * 128].rearrange(
                            "p (g d) -> p g d", g=gsz
                        ),
                        in0=gath[:, :gsz, :],
                        in1=val1_t[layer][:, g * GT1 : g * GT1 + gsz]
                        .unsqueeze(2)
                        .to_broadcast([128, gsz, 128]),
                        op=mybir.AluOpType.mult,
                    )
                    prod_of[g] = prod
                    prod_of.pop(g - 3, None)

                for fi, (q0, nq, fb, pbase) in enumerate(
                    [] if "nop1" in ablate else pl["fills1"]
                ):
                    ps = pps.tile([128, 512], f32, tag="ps", name=f"p1_{layer}_{fi}")
                    for q in range(q0, q0 + nq):
                        g = (4 * q) // GT1
                        while nextg[0] <= g:
                            issue_group1(nextg[0])
                            nextg[0] += 1
                        lo = 4 * q - g * GT1
                        nc.tensor.matmul(
                            out=ps[int(qp0[q]) : int(qp0[q] + qR[q]), :],
                            lhsT=lhs_t[
                                :, base1[int(qwi[q])] : base1[int(qwi[q])] + int(qR[q])
                            ],
                            rhs=prod_of[g][:, lo * 128 : (lo + 4) * 128],
                            start=True,
                            stop=True,
                            tile_position=(0, int(qp0[q])),
                        )
                    pb = pyo.tile([128, 512], bf16, tag="p1o", name=f"po_{layer}_{fi}")
                    nc.vector.tensor_copy(out=pb[:], in_=ps[:])
                    dst = part[
                        fb * WIN + pbase : fb * WIN + pbase + 512, :
                    ].rearrange("(c p) d -> p c d", c=4)
                    nc.sync.dma_start(
                        out=dst, in_=pb[:].rearrange("p (c d) -> p c d", c=4)
                    )

                # ---- phase 2 ----
                ydram = ydst[layer]
                if layer == 0:
                    for b2, bl in enumerate([] if "nop2" in ablate else _P2_BLOCKS):
                        ntil = bl["tiles"]
                        g2_of = {}
                        ng2done = [0]

                        def issue_group2(g, b2=b2, bl=bl, ntil=ntil, g2_of=g2_of):
                            gsz = min(GT2, ntil - g * GT2)
                            ixt = pix.tile(
                                [128, GT2 * 8], i16, tag="ix2",
                                name=f"ix2_{layer}_{b2}_{g}",
                            )
                            nc.sync.dma_start(
                                out=ixt[:, : gsz * 8],
                                in_=g2idx0[
                                    :,
                                    (bl["tbase"] + g * GT2) * 8 : (
                                        bl["tbase"] + g * GT2 + gsz
                                    )
                                    * 8,
                                ],
                            )
                            gt = pg2.tile(
                                [128, GT2, 128], bf16, tag="g2",
                                name=f"g2_{layer}_{b2}_{g}",
                            )
                            nc.gpsimd.dma_gather(
                                out_ap=gt[:, :gsz, :],
                                in_ap=part[b2 * WIN : (b2 + 1) * WIN, :],
                                idxs_ap=ixt[:, : gsz * 8],
                                num_idxs=gsz * 128,
                                num_idxs_reg=gsz * 128,
                                elem_size=128,
                                single_packet=False,
                                queue_num=next_q(gsz * 128),
                            )
                            g2_of[g] = gt
                            g2_of.pop(g - 3, None)

                        for f in range(bl["fills"]):
                            ps = pps.tile(
                                [128, 128], f32, tag="ps2", bufs=2,
                                name=f"p2_{layer}_{b2}_{f}",
                            )
                            for t in range(10):
                                tloc = 10 * f + t
                                g = tloc // GT2
                                while ng2done[0] <= g:
                                    issue_group2(ng2done[0])
                                    ng2done[0] += 1
                                lo = tloc - g * GT2
                                half = t // 5
                                pat = t % 5
                                nc.tensor.matmul(
                                    out=ps[half * 64 : half * 64 + 64, :],
                                    lhsT=lhs_t[
                                        :, base2 + pat * 64 : base2 + pat * 64 + 64
                                    ],
                                    rhs=g2_of[g][:, lo, :],
                                    start=(pat == 0),
                                    stop=(pat == 4),
                                    tile_position=(0, half * 64),
                                )
                            yb = bl["ybase"] + 128 * f
                            ybf = pyo.tile(
                                [128, 128], bf16, tag="ybf",
                                name=f"yb_{layer}_{b2}_{f}",
                            )
                            nc.vector.tensor_copy(out=ybf[:], in_=ps[:])
                            nc.sync.dma_start(
                                out=ydram[yb : yb + 128, :], in_=ybf[:]
                            )
                else:
                    p2 = plan["p2p"][layer]
                    for bl in [] if "nop2" in ablate else p2["blocks"]:
                        b2 = bl["b2"]
                        ntil = 10 * bl["fills"]
                        g2_of = {}
                        ng2done = [0]

                        def issue_group2p(
                            g, b2=b2, bl=bl, ntil=ntil, g2_of=g2_of, layer=layer
                        ):
                            gsz = min(GT2, ntil - g * GT2)
                            ixt = pix.tile(
                                [128, GT2 * 8], i16, tag="ix2",
                                name=f"ix2_{layer}_{b2}_{g}",
                            )
                            nc.sync.dma_start(
                                out=ixt[:, : gsz * 8],
                                in_=g2p[layer][
                                    :,
                                    (bl["tbase"] + g * GT2) * 8 : (
                                        bl["tbase"] + g * GT2 + gsz
                                    )
                                    * 8,
                                ],
                            )
                            gt = pg2.tile(
                                [128, GT2, 128], bf16, tag="g2",
                                name=f"g2_{layer}_{b2}_{g}",
                            )
                            nc.gpsimd.dma_gather(
                                out_ap=gt[:, :gsz, :],
                                in_ap=part[b2 * WIN : (b2 + 1) * WIN, :],
                                idxs_ap=ixt[:, : gsz * 8],
                                num_idxs=gsz * 128,
                                num_idxs_reg=gsz * 128,
                                elem_size=128,
                                single_packet=False,
                                queue_num=next_q(gsz * 128),
                            )
                            g2_of[g] = gt
                            g2_of.pop(g - 3, None)

                        for f in range(bl["fills"]):
                            ps = pps.tile(
                                [128, 128], f32, tag="ps2", bufs=2,
                                name=f"p2_{layer}_{b2}_{f}",
                            )
                            for t in range(10):
                                tloc = 10 * f + t
                                g = tloc // GT2
                                while ng2done[0] <= g:
                                    issue_group2p(ng2done[0])
                                    ng2done[0] += 1
                                lo = tloc - g * GT2
                                half = t // 5
                                pat = t % 5
                                nc.tensor.matmul(
                                    out=ps[half * 64 : half * 64 + 64, :],
                                    lhsT=lhs_t[
                                        :, base2 + pat * 64 : base2 + pat * 64 + 64
                                    ],
                                    rhs=g2_of[g][:, lo, :],
                                    start=(pat == 0),
                                    stop=(pat == 4),
                                    tile_position=(0, half * 64),
                                )
                            ybf = pyo.tile(
                                [128, 128], bf16, tag="ybf",
                                name=f"yb_{layer}_{b2}_{f}",
                            )
                            nc.vector.tensor_copy(out=ybf[:], in_=ps[:])
                            nc.gpsimd.indirect_dma_start(
                                out=ydram[:],
                                out_offset=bass.IndirectOffsetOnAxis(
                                    ap=yp_t[layer][:, bl["fbase"] + f : bl["fbase"] + f + 1],
                                    axis=0,
                                ),
                                in_=ybf[:],
                                in_offset=None,
                            )

                if layer < L - 1 and "noag" not in ablate:
                    nc.gpsimd.collective_compute(
                        "AllGather",
                        mybir.AluOpType.bypass,
                        replica_groups=[list(range(NCORES))],
                        ins=[ydram[:].opt()],
                        outs=[srcs[layer + 1][:].opt()],
                    )

            # ---- epilogue ----
            for kind, pos_t in (
                [] if "noepi" in ablate else (("u", bu_t), ("i", bi_t))
            ):
                for blk in range(NBB):
                    gego = pg2.tile([128, 128], f32, tag="bg", name=f"bg_{kind}_{blk}")
                    nc.gpsimd.indirect_dma_start(
                        out=gego[:],
                        out_offset=None,
                        in_=ego[:],
                        in_offset=bass.IndirectOffsetOnAxis(
                            ap=pos_t[:, blk : blk + 1], axis=0
                        ),
                    )
                    gy = []
                    for li, yt in enumerate(ydst):
                        g = pg2.tile(
                            [128, 128], bf16, tag=f"by{li}",
                            name=f"by{li}_{kind}_{blk}",
                        )
                        nc.gpsimd.indirect_dma_start(
                            out=g[:],
                            out_offset=None,
                            in_=yt[:],
                            in_offset=bass.IndirectOffsetOnAxis(
                                ap=pos_t[:, blk : blk + 1], axis=0
                            ),
                        )
                        gy.append(g)
                    s12 = pyo.tile([128, 128], f32, tag="s12", name=f"s12_{kind}_{blk}")
                    nc.vector.tensor_add(out=s12[:], in0=gy[0][:], in1=gy[1][:])
                    y3f = pyo.tile([128, 128], f32, tag="y3f", name=f"y3f_{kind}_{blk}")
                    nc.vector.tensor_copy(out=y3f[:], in_=gy[2][:])
                    s123 = pyo.tile(
                        [128, 128], f32, tag="s123", name=f"s123_{kind}_{blk}"
                    )
                    nc.vector.tensor_add(out=s123[:], in0=s12[:], in1=y3f[:])
                    allt = pyo.tile([128, 128], f32, tag="allt", name=f"al_{kind}_{blk}")
                    nc.vector.tensor_add(out=allt[:], in0=s123[:], in1=gego[:])
                    tps = pps.tile(
                        [64, 128], f32, tag="tps", bufs=1, name=f"tp_{kind}_{blk}"
                    )
                    nc.tensor.transpose(
                        out=tps[:], in_=allt[:, 0:64], identity=ident_t[:]
                    )
                    xt = pyo.tile([64, 128], f32, tag="xt", name=f"xt_{kind}_{blk}")
                    nc.vector.tensor_copy(out=xt[:], in_=tps[:])
                    pmm = pps.tile(
                        [128, 64], f32, tag="pmm", bufs=1, name=f"pm_{kind}_{blk}"
                    )
                    nc.tensor.matmul(
                        out=pmm[:], lhsT=xt[:], rhs=w4t_t[:], start=True, stop=True
                    )
                    prd = pyo.tile([128, 64], f32, tag="prd", name=f"pd_{kind}_{blk}")
                    nc.vector.tensor_add(out=prd[:], in0=pmm[:], in1=brep_t[:])
                    nc.sync.dma_start(
                        out=outs[f"{kind}_pred"][blk * 128 : (blk + 1) * 128, :],
                        in_=prd[:],
                    )
                    tgt = pyo.tile([128, 64], f32, tag="tgt", name=f"tg_{kind}_{blk}")
                    nc.vector.tensor_scalar_mul(
                        out=tgt[:], in0=allt[:, 64:128], scalar1=0.25
                    )
                    nc.sync.dma_start(
                        out=outs[f"{kind}_target"][blk * 128 : (blk + 1) * 128, :],
                        in_=tgt[:],
                    )

    nc.compile()
    return nc


def _wrap16(arr_128_nt):
    """[128, NT] lane/tile array -> [128, NT*8] int16 dma_gather wrap
    (position i = tile*128 + lane at [i%16, i//16]), replicated 8x."""
    flat = arr_128_nt.T.reshape(-1)
    w = np.ascontiguousarray(flat.reshape(-1, 16).T)
    return np.tile(w, (8, 1))


def _make_in_maps(plan, inputs):
    bf = ml_dtypes.bfloat16
    ego2 = np.concatenate(
        [
            np.concatenate(
                [inputs["user_emb_online"], inputs["item_emb_online"]], axis=0
            ),
            np.concatenate(
                [inputs["user_emb_target"], inputs["item_emb_target"]], axis=0
            ),
        ],
        axis=1,
    ).astype(np.float32)
    x2_0 = np.zeros((NP, 128), bf)
    ego_s = np.zeros((NCORES, S, 128), np.float32)
    for k in range(NCORES):
        x2_0[k * S : k * S + RPC] = ego2[k * RPC : (k + 1) * RPC].astype(bf)
        ego_s[k, :RPC] = ego2[k * RPC : (k + 1) * RPC]

    base1, base2, RSUM = _lhs_layout()
    lhs = np.zeros((128, RSUM), bf)
    for wi, w in enumerate(W1):
        for j in range(R1[wi]):
            lhs[j * w : (j + 1) * w, base1[wi] + j] = 1.0
    for pat in range(5):
        for l in range(128):
            m = (pat * 128 + l) // W2
            lhs[l, base2 + pat * 64 + m % 64] = 1.0

    w4t = np.ascontiguousarray(inputs["pred_w"].astype(np.float32).T * 0.25)
    brep = np.tile(inputs["pred_b"].astype(np.float32)[None, :], (128, 1))
    ident = np.eye(128, dtype=np.float32)

    in_maps = []
    for k in range(NCORES):
        m = {
            "x2_0": x2_0,
            "ego": ego_s[k],
            "g2idx0": _wrap16(plan["gidx2_full"][k]),
            "lhs": lhs,
            "w4t": w4t,
            "brep": brep,
            "ident": ident,
            "bupos": plan["bu_pos"][k],
            "bipos": plan["bi_pos"][k],
        }
        for l in range(L):
            pl = plan["layers"][l]
            m[f"g1idx_{l}"] = _wrap16(pl["gidx1"][k])
            m[f"val1_{l}"] = pl["val1"][k].astype(bf)
        for l in (1, 2):
            p2 = plan["p2p"][l]
            m[f"g2idx_{l}"] = _wrap16(p2["g2"][k])
            m[f"ypos_{l}"] = p2["ypos"][k]
        in_maps.append(m)
    return in_maps


_CACHE = {}
last_exec_ns = None
last_results = None


def kernel(**inputs):
    global last_exec_ns, last_results
    from concourse.bass_utils import run_bass_kernel_spmd

    inputs = {k: np.asarray(v) for k, v in inputs.items()}
    key = (
        inputs["adj_row"].tobytes()[:4096],
        inputs["adj_col"].tobytes()[:4096],
        inputs["user_idx"].tobytes(),
        inputs["item_idx"].tobytes(),
        int(inputs["adj_row"].shape[0]),
    )
    if key in _CACHE:
        plan, nc = _CACHE[key]
    else:
        plan = _plan(
            inputs["adj_row"],
            inputs["adj_col"],
            inputs["adj_val"],
            inputs["user_idx"],
            inputs["item_idx"],
        )
        plan = _plan_batch(plan, inputs["user_idx"], inputs["item_idx"])
        nc = _build_program(plan)
        _CACHE[key] = (plan, nc)

    in_maps = _make_in_maps(plan, inputs)
    global _LAST_IN_MAPS
    _LAST_IN_MAPS = in_maps
    res = run_bass_kernel_spmd(nc, in_maps, list(range(NCORES)))
    last_exec_ns = res.exec_time_ns
    last_results = res

    out = []
    for kind in ("u", "i"):
        owner = plan[f"b{kind}_owner"]
        slot = plan[f"b{kind}_slot"]
        pred = np.empty((B, 64), np.float32)
        tgt = np.empty((B, 64), np.float32)
        for k in range(NCORES):
            m = owner == k
            pred[m] = res.results[k][f"{kind}_pred"][slot[m]]
            tgt[m] = res.results[k][f"{kind}_target"][slot[m]]
        out.append((pred, tgt))
    return (out[0][0], out[0][1], out[1][0], out[1][1])


def measure_time(iters=4):
    """Wall-clock the compiled NEFF on warm device-resident inputs.
    Returns (best_seconds, all_times). Uses the cached plan/nc + last inputs
    (call kernel() first)."""
    import time as _time

    import jax
    from jax.sharding import Mesh, PartitionSpec
    from jax.experimental.shard_map import shard_map
    import concourse.mybir as mybir
    from concourse import bass2jax

    assert _CACHE, "call kernel() first"
    (plan, nc) = next(iter(_CACHE.values()))
    in_maps = _LAST_IN_MAPS
    assert in_maps is not None

    partition_name = nc.partition_id_tensor.name if nc.partition_id_tensor else None
    in_names, out_names, out_avals, zero_outs = [], [], [], []
    for alloc in nc.m.functions[0].allocations:
        if not isinstance(alloc, mybir.MemoryLocationSet):
            continue
        name = alloc.memorylocations[0].name
        if alloc.kind == "ExternalInput":
            if name != partition_name:
                in_names.append(name)
        elif alloc.kind == "ExternalOutput":
            out_names.append(name)
            shape = tuple(alloc.tensor_shape)
            dtype = mybir.dt.np(alloc.dtype)
            out_avals.append(jax.core.ShapedArray(shape, dtype))
            zero_outs.append(np.zeros(shape, dtype))
    n_params = len(in_names)
    n_outs = len(out_avals)
    all_in_names = list(in_names) + out_names
    if partition_name is not None:
        all_in_names.append(partition_name)

    def _body(*args):
        operands = list(args)
        if partition_name is not None:
            operands.append(bass2jax.partition_id_tensor())
        outs = bass2jax._bass_exec_p.bind(
            *operands,
            out_avals=tuple(out_avals),
            in_names=tuple(all_in_names),
            out_names=tuple(out_names),
            lowering_input_output_aliases=(),
            sim_require_finite=True,
            sim_require_nnan=True,
            nc=nc,
        )
        return tuple(outs)

    devices = jax.devices()[:NCORES]
    mesh = Mesh(np.asarray(devices), ("core",))
    donate = tuple(range(n_params, n_params + n_outs))
    sharded = jax.jit(
        shard_map(
            _body,
            mesh=mesh,
            in_specs=(PartitionSpec("core"),) * (n_params + n_outs),
            out_specs=(PartitionSpec("core"),) * n_outs,
            check_rep=False,
        ),
        donate_argnums=donate,
        keep_unused=True,
    )
    from jax.sharding import NamedSharding

    shard = NamedSharding(mesh, PartitionSpec("core"))
    concat_in = [
        jax.device_put(
            np.concatenate([np.asarray(in_maps[c][n]) for c in range(NCORES)], 0),
            shard,
        )
        for n in in_names
    ]
    times = []
    out = None
    for it in range(iters):
        zeros_dev = [
            jax.device_put(
                np.zeros((NCORES * z.shape[0], *z.shape[1:]), z.dtype), shard
            )
            for z in zero_outs
        ]
        jax.block_until_ready(zeros_dev)
        t0 = _time.perf_counter()
        out = sharded(*concat_in, *zeros_dev)
        jax.block_until_ready(out)
        times.append(_time.perf_counter() - t0)
    return min(times[1:]) if len(times) > 1 else times[0], times


_LAST_IN_MAPS = None
